# revision 1
# baseline (speedup 1.0000x reference)
"""LongcatFlash MoE kernel for 8 TRN2 NeuronCores (expert-parallel).

Contract: kernel(**inputs) takes the FULL un-sharded inputs from
reference.setup_inputs() and returns the FULL [T, H] output.

Strategy (expert-parallel, memory-regime):
  - Router runs replicated on every core in exact fp32 (top-4 selection
    gaps can be ~3e-7, so the logits matmul stays full-precision).
  - Experts are sharded 4 weight-slots per core by a host-computed static
    tile schedule (load-balancing metadata only; all routing, indices and
    gatings are computed on device). Hot experts are split across cores
    by token-rank ranges.
  - Each core: routes all tokens, builds per-tile dispatch slot lists
    (rank prefix-sums via tensor_tensor_scan + a strict-triangular
    matmul; slot->token inversion via a small one-hot matmul), gathers
    token rows with dma_gather, runs the two FFN matmuls, scales rows by
    gatings, and dma_scatter_add's them into a per-core partial output.
  - Zero-experts (ids >= 32) reduce to a per-token scale of the hidden
    row, applied by the token-range owner core.
  - Host unshards by summing the 8 partial outputs and undoing the row
    permutation r = (t % 128) * 16 + t // 128.
"""

import os
import numpy as np

import concourse.bacc as bacc
import concourse.bass as bass
import concourse.mybir as mybir
import concourse.tile as tile
from concourse import library_config
from concourse.bass_utils import run_bass_kernel_spmd

F32 = mybir.dt.float32
F32R = mybir.dt.float32r
BF16 = mybir.dt.bfloat16
I16 = mybir.dt.int16
U32 = mybir.dt.uint32
U8 = mybir.dt.uint8

T, H, I = 2048, 1024, 512
NE, ER, TOPK = 40, 32, 4
ROUTED_SCALE = 2.5
NCORES = 8
NJ = T // 128              # 16 token tiles (r = p*16 + j)
TMAX = 10                  # static FFN tiles per core
NSL = 5                    # weight slots per core
SLOT_OF_TILE = [0, 0, 0, 0, 1, 1, 2, 2, 3, 4]   # tile -> local weight slot
SLOT_CAP = [4, 2, 2, 1, 1]
SLOT_TILES = [[0, 1, 2, 3], [4, 5], [6, 7], [8], [9]]
NSLOT = TMAX * 128         # 1280 dispatch slots per core
AluOp = mybir.AluOpType
ACT_F = mybir.ActivationFunctionType
AXL = mybir.AxisListType

# FFN matmul dtype: "f32r" (fp32 data, full-rate PE mode) or "bf16".
FFN_DTYPE = os.environ.get("MOE_FFN_DTYPE", "f32r")


# ---------------------------------------------------------------------------
# host-side schedule
# ---------------------------------------------------------------------------

def _host_routing(hidden, router_w, bias):
    """fp32 routing on host — used ONLY for load-balance scheduling."""
    logits = hidden.astype(np.float32) @ router_w.astype(np.float32).T
    m = logits.max(axis=1, keepdims=True)
    e = np.exp(logits - m)
    scores = e / e.sum(axis=1, keepdims=True)
    biased = scores + bias[None, :]
    ids = np.argsort(-biased, axis=1, kind="stable")[:, :TOPK]
    return ids


def _schedule(ids):
    """Static tile schedule: split-anywhere first-fit-decreasing packing.

    Returns per-core:
      slot_expert[c][s]: global expert id serviced by local weight slot s
      tiles[c][tau]: (expert_id, lo_rank) — dispatch range for FFN tile tau
    Ranks are positions within an expert's selected-token list in r-order.
    """
    counts = np.zeros(ER, np.int64)
    for row in ids:
        for e in row:
            if e < ER:
                counts[e] += 1
    pieces = [[e, 0, (int(counts[e]) + 127) // 128] for e in range(ER)
              if counts[e] > 0]               # [expert, first_tile, ntiles]
    pieces.sort(key=lambda p: -p[2])
    slots = sorted(((SLOT_CAP[s], c, s) for c in range(NCORES)
                    for s in range(NSL)), key=lambda x: -x[0])
    slot_expert = [[0] * NSL for _ in range(NCORES)]
    tiles = [[(0, 1 << 14)] * TMAX for _ in range(NCORES)]
    si = 0
    work = []
    for p in pieces:
        work.append(p)
    while work:
        work.sort(key=lambda p: -p[2])
        p = work.pop(0)
        if si >= len(slots):
            raise RuntimeError("schedule: out of weight slots")
        cap, c, s = slots[si]
        si += 1
        take = min(cap, p[2])
        slot_expert[c][s] = p[0]
        for k in range(cap):
            tau = SLOT_TILES[s][k]
            # tiles beyond `take` extend the range as harmless slack
            tiles[c][tau] = (p[0], 128 * (p[1] + min(k, take)))
        for k in range(take):
            tiles[c][SLOT_TILES[s][k]] = (p[0], 128 * (p[1] + k))
        if p[2] > take:
            work.append([p[0], p[1] + take, p[2] - take])
    return slot_expert, tiles


# ---------------------------------------------------------------------------
# device graph
# ---------------------------------------------------------------------------

_NC_CACHE = {}


def build_nc():
    key = FFN_DTYPE
    if key in _NC_CACHE:
        return _NC_CACHE[key]
    nc = bacc.Bacc("TRN2", target_bir_lowering=False, debug=False,
                   num_devices=NCORES)

    def din(name, shape, dt):
        return nc.dram_tensor(name, shape, dt, kind="ExternalInput").ap()

    hidden_T = din("hidden_T", [H, T], F32)            # original token order
    hidden_rows = din("hidden_rows", [T, H], F32)      # r-ordered rows
    rwt = din("rwt", [H, NE], F32)                     # router_w.T
    bias_b = din("bias_b", [128, NE], F32)             # bias replicated
    w13s = din("w13s", [NSL, H, 2 * I], F32R)             # per-slot [h, i]
    w2s = din("w2s", [NSL, I, H], F32R)                   # per-slot [i, h]
    tile_e = din("tile_e", [128, TMAX], F32)           # expert id per tile
    tile_lo = din("tile_lo", [128, TMAX], F32)         # rank range lo per tile
    rhl = din("rhl", [128, NJ, 2], BF16)               # r split (r//128, r%128)
    iota128 = din("iota128", [128, 128], F32)          # row 0..127 replicated
    ident = din("ident", [128, 128], F32)
    identb = din("identb", [128, 128], BF16)
    uts128 = din("uts128", [128, 128], F32)            # strict upper: [k,m]=k<m
    hz = din("hz", [256, H], F32)                      # my zero-path rows
    seqidx = din("seqidx", [128, 16], I16)             # my zero-path idxs

    partial = nc.dram_tensor("partial", [T, H], F32, kind="ExternalOutput").ap()

    bf = FFN_DTYPE == "bf16"
    xdt = BF16 if bf else F32R

    with tile.TileContext(nc) as tc:
        with (
            tc.tile_pool(name="const", bufs=1) as cpool,
            tc.tile_pool(name="work", bufs=2) as wpool,
            tc.tile_pool(name="persist", bufs=1) as ppool,
            tc.tile_pool(name="wload", bufs=2) as wlpool,
            tc.tile_pool(name="wloadx", bufs=1) as wxpool,
            tc.tile_pool(name="psum", bufs=2, space="PSUM") as pspool,
            tc.tile_pool(name="psumS", bufs=1, space="PSUM") as psspool,
            tc.tile_pool(name="psumA", bufs=1, space="PSUM") as psapool,
            tc.tile_pool(name="dram", bufs=1, space="DRAM") as dpool,
        ):
            nc.gpsimd.load_library(library_config.mlp)

            # ---- resident constants ----
            rw_sb = cpool.tile([128, 8, NE], F32, tag="rw")
            nc.sync.dma_start(rw_sb[:], rwt.rearrange("(k p) n -> p k n", p=128))
            bias_sb = cpool.tile([128, NE], F32, tag="bias")
            nc.sync.dma_start(bias_sb[:], bias_b[:])
            iota_sb = cpool.tile([128, 128], F32, tag="iota")
            nc.sync.dma_start(iota_sb[:], iota128[:])
            ident_sb = cpool.tile([128, 128], F32, tag="ident")
            nc.sync.dma_start(ident_sb[:], ident[:])
            identb_sb = cpool.tile([128, 128], BF16, tag="identb")
            nc.sync.dma_start(identb_sb[:], identb[:])
            identr_sb = cpool.tile([128, 128], F32R, tag="identr")
            nc.vector.tensor_copy(identr_sb[:], ident_sb[:])
            uts_sb = cpool.tile([128, 128], F32, tag="uts")
            nc.sync.dma_start(uts_sb[:], uts128[:])
            te_sb = cpool.tile([128, TMAX], F32, tag="te")
            nc.sync.dma_start(te_sb[:], tile_e[:])
            tlo_sb = cpool.tile([128, TMAX], F32, tag="tlo")
            nc.sync.dma_start(tlo_sb[:], tile_lo[:])
            zeros16 = cpool.tile([128, NJ], F32, tag="z16")
            nc.vector.memset(zeros16[:], 0.0)
            negbig = cpool.tile([128, NE], F32, tag="negbig")
            nc.vector.memset(negbig[:], -1e30)

            # ---- persistent intermediates ----
            idf_all = ppool.tile([128, NJ, TOPK], F32, tag="idf")
            g_all = ppool.tile([128, NJ, TOPK], F32, tag="gall")
            zt_all = ppool.tile([128, NJ], F32, tag="zt")
            lhsT_all = ppool.tile([128, NJ, 2 + TMAX], BF16, tag="lhsT")
            nc.sync.dma_start(lhsT_all[:, :, 0:2], rhl[:])

            # =============== phase 1: router (exact fp32) ===============
            # logits.T chunks: [40, 512] = rwt (stationary) x hidden_T slices
            for j in range(NJ):
                cq, q = j // 2, j % 2
                if q == 0:
                    ht = wpool.tile([128, 8, 256], F32, tag="ht")
                    nc.sync.dma_start(
                        ht[:],
                        hidden_T.rearrange("(k p) t -> p k t", p=128)[
                            :, :, cq * 256:(cq + 1) * 256])
                    ps_lg = psspool.tile([40, 256], F32, tag="ps_small",
                                         name="ps_lg")
                    for k in range(8):
                        nc.tensor.matmul(ps_lg[:], lhsT=rw_sb[:, k, :],
                                         rhs=ht[:, k, :],
                                         start=(k == 0), stop=(k == 7))
                    lgs = wpool.tile([40, 256], F32, tag="lgs")
                    nc.vector.tensor_copy(lgs[:], ps_lg[:])
                ps_l = pspool.tile([128, 128], F32, tag="ps_tr", name="ps_lt")
                nc.tensor.transpose(ps_l[:, :NE],
                                    lgs[:, q * 128:(q + 1) * 128],
                                    ident_sb[:NE, :NE])
                ps_l = ps_l[:, :NE]
                rmax = wpool.tile([128, 1], F32, tag="rmax")
                nc.vector.tensor_reduce(rmax[:], ps_l, axis=AXL.X,
                                        op=AluOp.max, negate=True)  # -max
                ex = wpool.tile([128, NE], F32, tag="ex")
                nc.scalar.activation(ex[:], ps_l, ACT_F.Exp, bias=rmax[:, 0:1])
                rsum = wpool.tile([128, 1], F32, tag="rsum")
                nc.vector.tensor_reduce(rsum[:], ex[:], axis=AXL.X, op=AluOp.add)
                rinv = wpool.tile([128, 1], F32, tag="rinv")
                nc.vector.reciprocal(rinv[:], rsum[:])
                scores = wpool.tile([128, NE], F32, tag="scores")
                nc.vector.tensor_scalar(scores[:], ex[:], rinv[:, 0:1], None,
                                        op0=AluOp.mult)
                biased = wpool.tile([128, NE], F32, tag="biased")
                nc.vector.tensor_add(biased[:], scores[:], bias_sb[:])
                top8 = wpool.tile([128, 8], F32, tag="top8")
                nc.vector.max(top8[:], biased[:])
                nc.vector.memset(top8[:, 4:8], -1e30)
                scr = wpool.tile([128, NE], F32, tag="scr")
                nc.vector.match_replace(scr[:], top8[:], biased[:], -1e30)
                pred = wpool.tile([128, NE], U8, tag="pred")
                nc.vector.tensor_tensor(pred[:], biased[:], scr[:], op=AluOp.is_gt)
                masked = wpool.tile([128, NE], F32, tag="masked")
                nc.vector.tensor_copy(masked[:], negbig[:])
                nc.vector.copy_predicated(masked[:], pred[:], scores[:])
                vals8 = wpool.tile([128, 8], F32, tag="vals8")
                nc.vector.max(vals8[:], masked[:])
                idx8 = wpool.tile([128, 8], U32, tag="idx8")
                nc.vector.max_index(idx8[:], vals8[:], masked[:])
                nc.vector.tensor_scalar(g_all[:, j, :], vals8[:, :TOPK],
                                        ROUTED_SCALE, None, op0=AluOp.mult)
                nc.vector.tensor_copy(idf_all[:, j, :], idx8[:, :TOPK])
                zm = wpool.tile([128, TOPK], F32, tag="zm")
                nc.vector.tensor_scalar(zm[:], idf_all[:, j, :], ER - 0.5, None,
                                        op0=AluOp.is_gt)
                nc.vector.tensor_mul(zm[:], zm[:], g_all[:, j, :])
                nc.vector.tensor_reduce(zt_all[:, j:j + 1], zm[:], axis=AXL.X,
                                        op=AluOp.add)

            zt_flat = dpool.tile([1, T], F32, tag="ztflat")
            nc.sync.dma_start(zt_flat[0, :].rearrange("(p j) -> p j", p=128),
                              zt_all[:])

            # =============== phase 2: dispatch bookkeeping ===============
            inv_ps = psapool.tile([2 + TMAX, NSLOT], F32, tag="ps_inv")
            slotm_tiles = []
            for tau in range(TMAX):
                eq = wpool.tile([128, NJ, TOPK], F32, tag="eq")
                nc.vector.tensor_tensor(
                    eq[:], idf_all[:],
                    te_sb[:, tau:tau + 1].to_broadcast([128, NJ, TOPK]),
                    op=AluOp.is_equal)
                gv = wpool.tile([128, NJ, TOPK], F32, tag="gvx")
                nc.vector.tensor_mul(gv[:], eq[:], g_all[:])
                mask = wpool.tile([128, NJ], F32, tag="mask")
                nc.vector.tensor_reduce(mask[:], eq[:], axis=AXL.X, op=AluOp.max)
                gval = wpool.tile([128, NJ], F32, tag="gval")
                nc.vector.tensor_reduce(gval[:], gv[:], axis=AXL.X, op=AluOp.add)
                nc.vector.tensor_copy(lhsT_all[:, :, 2 + tau], gval[:])
                rowsum = wpool.tile([128, 1], F32, tag="rowsum")
                nc.vector.tensor_reduce(rowsum[:], mask[:], axis=AXL.X,
                                        op=AluOp.add)
                inrow = wpool.tile([128, NJ], F32, tag="inrow")
                nc.vector.tensor_tensor_scan(inrow[:], mask[:], zeros16[:], 0.0,
                                             op0=AluOp.add, op1=AluOp.add)
                ps_rp = psspool.tile([128, 1], F32, tag="ps_small")
                nc.tensor.matmul(ps_rp[:], lhsT=uts_sb[:], rhs=rowsum[:],
                                 start=True, stop=True)
                pos = wpool.tile([128, NJ], F32, tag="pos")
                nc.vector.tensor_scalar(pos[:], inrow[:], ps_rp[:, 0:1], None,
                                        op0=AluOp.add)
                nc.vector.tensor_sub(pos[:], pos[:], mask[:])
                t1 = wpool.tile([128, NJ], F32, tag="t1")
                nc.vector.tensor_scalar(t1[:], pos[:], tlo_sb[:, tau:tau + 1],
                                        None, op0=AluOp.subtract)
                okr = wpool.tile([128, NJ], F32, tag="okr")
                nc.vector.tensor_scalar(okr[:], t1[:], -0.5, None, op0=AluOp.is_gt)
                ok2 = wpool.tile([128, NJ], F32, tag="ok2")
                nc.vector.tensor_scalar(ok2[:], t1[:], 127.5, None, op0=AluOp.is_lt)
                nc.vector.tensor_mul(okr[:], okr[:], ok2[:])
                nc.vector.tensor_mul(okr[:], okr[:], mask[:])
                oku = wpool.tile([128, NJ], U8, tag="oku")
                nc.vector.tensor_copy(oku[:], okr[:])
                t1m = ppool.tile([128, NJ], F32, tag=f"t1m{tau}")
                nc.vector.memset(t1m[:], -4.0)
                nc.vector.copy_predicated(t1m[:], oku[:], t1[:])
                slotm_tiles.append(t1m)

            for j in range(NJ):
                oh = wpool.tile([128, TMAX, 128], BF16, tag="oh")
                for tau in range(TMAX):
                    nc.vector.tensor_tensor(
                        oh[:, tau, :], iota_sb[:],
                        slotm_tiles[tau][:, j:j + 1].to_broadcast([128, 128]),
                        op=AluOp.is_equal)
                ohf = oh[:].rearrange("p a b -> p (a b)")
                for lo in range(0, NSLOT, 512):
                    hi = min(lo + 512, NSLOT)
                    nc.tensor.matmul(inv_ps[:, lo:hi], lhsT=lhsT_all[:, j, :],
                                     rhs=ohf[:, lo:hi],
                                     start=(j == 0), stop=(j == NJ - 1))

            # decode via PE transpose: tsp[p, tau, :] = inv_ps[:, tau*128+p]
            inv_sb = wpool.tile([2 + TMAX, NSLOT], F32, tag="invsb")
            nc.vector.tensor_copy(inv_sb[:], inv_ps[:])
            tsp_sb = ppool.tile([128, TMAX, 2 + TMAX], F32, tag="tsp")
            for tau in range(TMAX):
                ps_tsp = pspool.tile([128, 128], F32, tag="ps_tr", name="ps_tsp")
                nc.tensor.transpose(ps_tsp[:, :2 + TMAX],
                                    inv_sb[:, tau * 128:(tau + 1) * 128],
                                    ident_sb[:2 + TMAX, :2 + TMAX])
                nc.vector.tensor_copy(tsp_sb[:, tau, :], ps_tsp[:, :2 + TMAX])
            # r = 128*hi + lo  (per-partition layout [p, tau])
            r_pt = ppool.tile([128, TMAX], F32, tag="rpt")
            nc.vector.scalar_tensor_tensor(r_pt[:], tsp_sb[:, :, 0], 128.0,
                                           tsp_sb[:, :, 1], op0=AluOp.mult,
                                           op1=AluOp.add)
            g_wr = ppool.tile([128, TMAX], F32, tag="gwr")
            for tau in range(TMAX):
                nc.vector.tensor_copy(g_wr[:, tau:tau + 1],
                                      tsp_sb[:, tau, 2 + tau:3 + tau])
            # int16 slot->token list to DRAM, reload 16-wrapped for the DGE ops
            r_i16 = wpool.tile([128, TMAX], I16, tag="ri16")
            nc.vector.tensor_copy(r_i16[:], r_pt[:])
            slots_dram = dpool.tile([1, NSLOT], I16, tag="slotsdram")
            nc.sync.dma_start(
                slots_dram[0, :].rearrange("(t p) -> p t", p=128), r_i16[:])
            idxw = ppool.tile([128, NSLOT // 16], I16, tag="idxw")
            for grp in range(8):
                nc.sync.dma_start(
                    idxw[grp * 16:(grp + 1) * 16, :],
                    slots_dram[0, :].rearrange("(c p) -> p c", p=16))

            # =============== phase 3: gather + FFN + combine ===============
            for s in range(NSL):
                nt = SLOT_CAP[s]
                t0 = SLOT_TILES[s][0]
                x_sl = wxpool.tile([128, 4, H], F32, tag="xsl")
                nc.gpsimd.dma_gather(
                    out_ap=x_sl[:, :nt, :], in_ap=hidden_rows[:],
                    idxs_ap=idxw[:, t0 * 8:(t0 + nt) * 8],
                    num_idxs=nt * 128, num_idxs_reg=nt * 128, elem_size=H)
                w13_sb = wlpool.tile([128, 8, 2 * I], F32R, tag="w13")
                w2_sb = wlpool.tile([128, 4, H], F32R, tag="w2")
                nc.sync.dma_start(w13_sb[:],
                                  w13s[s].rearrange("(k p) i -> p k i", p=128))
                nc.sync.dma_start(w2_sb[:],
                                  w2s[s].rearrange("(k p) i -> p k i", p=128))
                if bf:
                    w13_b = wxpool.tile([128, 8, 2 * I], BF16, tag="w13b")
                    w2_b = wxpool.tile([128, 4, H], BF16, tag="w2b")
                    for k in range(8):
                        nc.scalar.activation(w13_b[:, k, :], w13_sb[:, k, :],
                                             ACT_F.Copy)
                    for k in range(4):
                        nc.scalar.activation(w2_b[:, k, :], w2_sb[:, k, :],
                                             ACT_F.Copy)

                for ti, tau in enumerate(SLOT_TILES[s]):
                    # transpose x rows: [slot, h] -> [h, slot]
                    if bf:
                        xc = wpool.tile([128, H], BF16, tag="xc")
                        nc.vector.tensor_copy(xc[:], x_sl[:, ti, :])
                    xt = wpool.tile([128, 8, 128], xdt, tag="xt")
                    for k in range(8):
                        ps_t = pspool.tile([128, 128], F32, tag="ps_tr")
                        src = xc[:, k * 128:(k + 1) * 128] if bf else \
                            x_sl[:, ti, k * 128:(k + 1) * 128]
                        idn = identb_sb if bf else ident_sb
                        nc.tensor.transpose(ps_t[:], src, idn[:])
                        nc.vector.tensor_copy(xt[:, k, :], ps_t[:])
                    # mm1
                    ps_gu = psapool.tile([128, 2 * I], F32, tag="ps_big")
                    for k in range(8):
                        for n in range(2):
                            if bf:
                                lhs1 = xt[:, k, :]
                                rhs1 = w13_b[:, k, n * 512:(n + 1) * 512]
                            else:
                                lhs1 = xt[:, k, :]
                                rhs1 = w13_sb[:, k, n * 512:(n + 1) * 512]
                            nc.tensor.matmul(ps_gu[:, n * 512:(n + 1) * 512],
                                             lhsT=lhs1, rhs=rhs1,
                                             start=(k == 0), stop=(k == 7))
                    # h = silu(gate) * up = gate * sigmoid(gate) * up
                    sg = wpool.tile([128, I], F32, tag="sg")
                    nc.scalar.activation(sg[:], ps_gu[:, :I], ACT_F.Sigmoid)
                    nc.vector.tensor_mul(sg[:], sg[:], ps_gu[:, :I])
                    hh = wpool.tile([128, I], xdt, tag="hh")
                    nc.vector.tensor_mul(hh[:], sg[:], ps_gu[:, I:])
                    ht2 = wpool.tile([128, 4, 128], xdt, tag="ht2")
                    for k in range(4):
                        ps_t2 = pspool.tile([128, 128], F32, tag="ps_tr")
                        idn = identb_sb if bf else identr_sb
                        pt2 = ps_t2[:] if bf else ps_t2[:].bitcast(F32R)
                        nc.tensor.transpose(pt2, hh[:, k * 128:(k + 1) * 128],
                                            idn[:])
                        nc.vector.tensor_copy(ht2[:, k, :], ps_t2[:])
                    # mm2
                    ps_y = psapool.tile([128, H], F32, tag="ps_big")
                    for k in range(4):
                        for n in range(2):
                            if bf:
                                lhs2 = ht2[:, k, :]
                                rhs2 = w2_b[:, k, n * 512:(n + 1) * 512]
                            else:
                                lhs2 = ht2[:, k, :]
                                rhs2 = w2_sb[:, k, n * 512:(n + 1) * 512]
                            nc.tensor.matmul(ps_y[:, n * 512:(n + 1) * 512],
                                             lhsT=lhs2, rhs=rhs2,
                                             start=(k == 0), stop=(k == 3))
                    yv = wpool.tile([128, 1, H], F32, tag="yv")
                    nc.scalar.activation(yv[:, 0, :], ps_y[:], ACT_F.Copy,
                                         scale=g_wr[:, tau:tau + 1])
                    nc.gpsimd.dma_scatter_add(
                        out_ap=partial[:], in_ap=yv[:],
                        idxs_ap=idxw[:, tau * 8:(tau + 1) * 8],
                        num_idxs=128, num_idxs_reg=128, elem_size=H)

            # =============== phase 4: zero-expert path ===============
            seq_sb = cpool.tile([128, 16], I16, tag="seqsb")
            nc.sync.dma_start(seq_sb[:], seqidx[:])
            pid = nc.sync.partition_id()
            for tt in range(2):
                ztv = wpool.tile([1, 128], F32, tag="ztv")
                nc.sync.dma_start(
                    ztv[:], zt_flat[0:1, bass.ds(pid * 256 + tt * 128, 128)])
                ps_zt = psspool.tile([128, 1], F32, tag="ps_small")
                nc.tensor.transpose(ps_zt[:], ztv[:], ident_sb[:1, :1])
                ztc = wpool.tile([128, 1], F32, tag="ztc")
                nc.vector.tensor_copy(ztc[:], ps_zt[:])
                hzt = wpool.tile([128, H], F32, tag="hzt")
                nc.sync.dma_start(hzt[:], hz[tt * 128:(tt + 1) * 128, :])
                yz = wpool.tile([128, 1, H], F32, tag="yz")
                nc.scalar.activation(yz[:, 0, :], hzt[:], ACT_F.Copy,
                                     scale=ztc[:, 0:1])
                nc.gpsimd.dma_scatter_add(
                    out_ap=partial[:], in_ap=yz[:],
                    idxs_ap=seq_sb[:, tt * 8:(tt + 1) * 8],
                    num_idxs=128, num_idxs_reg=128, elem_size=H)

    nc.compile()
    _NC_CACHE[key] = nc
    return nc


# ---------------------------------------------------------------------------
# host wrapper
# ---------------------------------------------------------------------------

def make_in_maps(hidden_states, router_w, e_score_correction_bias, w13, w2):
    import ml_dtypes
    hidden_states = np.asarray(hidden_states, np.float32)
    router_w = np.asarray(router_w, np.float32)
    bias = np.asarray(e_score_correction_bias, np.float32)
    w13 = np.asarray(w13, np.float32)
    w2 = np.asarray(w2, np.float32)

    ids = _host_routing(hidden_states, router_w, bias)
    slot_expert, tiles = _schedule(ids)

    # r = (t % 128) * 16 + t // 128  <->  t = (r % 16) * 128 + r // 16
    r_of_t = (np.arange(T) % 128) * 16 + np.arange(T) // 128
    t_of_r = np.empty(T, np.int64)
    t_of_r[r_of_t] = np.arange(T)

    hidden_T = np.ascontiguousarray(hidden_states.T)
    hidden_rows = np.ascontiguousarray(hidden_states[t_of_r])
    rwt = np.ascontiguousarray(router_w.T)
    bias_b = np.tile(bias[None, :], (128, 1))
    w13t = np.ascontiguousarray(w13.transpose(0, 2, 1))   # [e, h, 2I]
    w2t = np.ascontiguousarray(w2.transpose(0, 2, 1))     # [e, i, h]

    rr = np.arange(T).reshape(128, NJ).astype(np.float32)  # r at [p, j]
    rhl = np.stack([rr // 128, rr % 128], axis=-1).astype(ml_dtypes.bfloat16)
    iota128 = np.tile(np.arange(128, dtype=np.float32), (128, 1))
    ident = np.eye(128, dtype=np.float32)
    identb = np.eye(128, dtype=ml_dtypes.bfloat16)
    uts128 = np.triu(np.ones((128, 128), np.float32), k=1)

    p_ = np.arange(128)[:, None]
    f_ = np.arange(16)[None, :]
    seq_base = (f_ % 8) * 16 + (p_ % 16) + (f_ // 8) * 128  # [p, f]

    in_maps = []
    for c in range(NCORES):
        te = np.array([tiles[c][tau][0] for tau in range(TMAX)], np.float32)
        tlo = np.array([tiles[c][tau][1] for tau in range(TMAX)], np.float32)
        in_maps.append({
            "hidden_T": hidden_T,
            "hidden_rows": hidden_rows,
            "rwt": rwt,
            "bias_b": bias_b,
            "w13s": np.ascontiguousarray(
                w13t[[slot_expert[c][s] for s in range(NSL)]]),
            "w2s": np.ascontiguousarray(
                w2t[[slot_expert[c][s] for s in range(NSL)]]),
            "tile_e": np.tile(te[None, :], (128, 1)),
            "tile_lo": np.tile(tlo[None, :], (128, 1)),
            "rhl": rhl,
            "iota128": iota128,
            "ident": ident,
            "identb": identb,
            "uts128": uts128,
            "hz": np.ascontiguousarray(hidden_rows[c * 256:(c + 1) * 256]),
            "seqidx": (seq_base + c * 256).astype(np.int16),
        })
    return in_maps, t_of_r


def kernel(hidden_states, router_w, e_score_correction_bias, w13, w2,
           _trace=False):
    nc = build_nc()
    in_maps, t_of_r = make_in_maps(hidden_states, router_w,
                                   e_score_correction_bias, w13, w2)
    res = run_bass_kernel_spmd(nc, in_maps, core_ids=list(range(NCORES)),
                               trace=_trace)
    total = np.zeros((T, H), np.float64)
    for c in range(NCORES):
        total += res.results[c]["partial"].astype(np.float64)
    out = np.empty((T, H), np.float32)
    out[t_of_r] = total.astype(np.float32)      # out[t] = total[r(t)]
    kernel._last_results = res
    return out



# revision 4
# speedup vs baseline: 1.2457x; 1.2457x over previous
"""LongcatFlash MoE kernel for 8 TRN2 NeuronCores (expert-parallel).

Contract: kernel(**inputs) takes the FULL un-sharded inputs from
reference.setup_inputs() and returns the FULL [T, H] output.

Strategy (expert-parallel, memory-regime), v2:
  - Router runs replicated on every core in exact fp32; the whole
    softmax / top-4 selection pipeline is batched over all 16 token
    tiles as [128, 16, 40] 3D vector ops (per-tile work is only the
    top-8 instruction that yields the 4th-largest threshold).
  - Selection is a mask (biased >= 4th-largest); gates flow as the
    masked-scaled score vector (40 wide) straight into the dispatch
    inversion matmul, so no top-k id/gate extraction is needed.
  - Experts are sharded 4-5 weight-slots per core by a host-computed
    static tile schedule (load-balancing metadata only; all routing is
    on device). Hot experts split across cores by token-rank ranges.
  - Dispatch: per-tile rank prefix-sums (scan + strict-triangular
    matmul), one bf16 one-hot is_eq per token tile, inversion matmuls
    with a 42-wide lhsT [r_hi, r_lo, gates(40)], PE-transpose decode.
    The slot->token index list is built on-chip with small permutation
    matmuls (no DRAM round-trip).
  - FFN: bf16 weights DMA'd directly; token rows gathered via
    dma_gather(transpose=True) from a bf16 copy of hidden, landing
    pre-transposed for mm1 (no x transposes). Only the 4 h-transposes
    per tile remain on the PE. Output rows scaled by slot gates and
    dma_scatter_add'ed into a per-core partial output.
  - Zero-experts (ids >= 32) reduce to a per-token scale of the hidden
    row, applied by the token-range owner core.
  - Host unshards by summing the 8 partial outputs and undoing the row
    permutation r = (t % 128) * 16 + t // 128.
"""

import numpy as np

import concourse.bacc as bacc
import concourse.bass as bass
import concourse.mybir as mybir
import concourse.tile as tile
from concourse import library_config
from concourse.bass_utils import run_bass_kernel_spmd

F32 = mybir.dt.float32
BF16 = mybir.dt.bfloat16
I16 = mybir.dt.int16
U8 = mybir.dt.uint8

T, H, I = 2048, 1024, 512
NE, ER = 40, 32
ROUTED_SCALE = 2.5
NCORES = 8
NJ = T // 128              # 16 token tiles (r = p*16 + j)
TMAX = 10                  # static FFN tiles per core
NSL = 5                    # weight slots per core
SLOT_CAP = [4, 2, 2, 1, 1]
SLOT_TILES = [[0, 1, 2, 3], [4, 5], [6, 7], [8], [9]]
NSLOT = TMAX * 128         # 1280 dispatch slots per core
LW = 2 + NE                # inversion lhsT width: r_hi, r_lo, 40 gates
AluOp = mybir.AluOpType
ACT_F = mybir.ActivationFunctionType
AXL = mybir.AxisListType


# ---------------------------------------------------------------------------
# host-side schedule
# ---------------------------------------------------------------------------

def _host_routing(hidden, router_w, bias):
    """fp32 routing on host — used ONLY for load-balance scheduling."""
    logits = hidden.astype(np.float32) @ router_w.astype(np.float32).T
    m = logits.max(axis=1, keepdims=True)
    e = np.exp(logits - m)
    scores = e / e.sum(axis=1, keepdims=True)
    biased = scores + bias[None, :]
    ids = np.argsort(-biased, axis=1, kind="stable")[:, :4]
    return ids


def _schedule(ids):
    """Static tile schedule: split-anywhere first-fit-decreasing packing.

    Returns per-core:
      slot_expert[c][s]: global expert id serviced by local weight slot s
      tiles[c][tau]: (expert_id, lo_rank) — dispatch range for FFN tile tau
    Ranks are positions within an expert's selected-token list in r-order.
    """
    counts = np.zeros(ER, np.int64)
    for row in ids:
        for e in row:
            if e < ER:
                counts[e] += 1
    pieces = [[e, 0, (int(counts[e]) + 127) // 128] for e in range(ER)
              if counts[e] > 0]               # [expert, first_tile, ntiles]
    pieces.sort(key=lambda p: -p[2])
    slots = sorted(((SLOT_CAP[s], c, s) for c in range(NCORES)
                    for s in range(NSL)), key=lambda x: -x[0])
    slot_expert = [[0] * NSL for _ in range(NCORES)]
    tiles = [[(0, 1 << 14)] * TMAX for _ in range(NCORES)]
    si = 0
    work = []
    for p in pieces:
        work.append(p)
    while work:
        work.sort(key=lambda p: -p[2])
        p = work.pop(0)
        if si >= len(slots):
            raise RuntimeError("schedule: out of weight slots")
        cap, c, s = slots[si]
        si += 1
        take = min(cap, p[2])
        slot_expert[c][s] = p[0]
        for k in range(cap):
            tau = SLOT_TILES[s][k]
            # tiles beyond `take` extend the range as harmless slack
            tiles[c][tau] = (p[0], 128 * (p[1] + min(k, take)))
        for k in range(take):
            tiles[c][SLOT_TILES[s][k]] = (p[0], 128 * (p[1] + k))
        if p[2] > take:
            work.append([p[0], p[1] + take, p[2] - take])
    return slot_expert, tiles


# ---------------------------------------------------------------------------
# device graph
# ---------------------------------------------------------------------------

_NC_CACHE = {}


def build_nc():
    key = "v2"
    if key in _NC_CACHE:
        return _NC_CACHE[key]
    nc = bacc.Bacc("TRN2", target_bir_lowering=False, debug=False,
                   num_devices=NCORES)

    def din(name, shape, dt):
        return nc.dram_tensor(name, shape, dt, kind="ExternalInput").ap()

    hidden_T = din("hidden_T", [H, T], F32)            # original token order
    hidden_bf = din("hidden_bf", [T, H], BF16)         # r-ordered rows, bf16
    rwt = din("rwt", [H, NE], F32)                     # router_w.T
    bias_b = din("bias_b", [128, NE], F32)             # bias replicated
    w13s = din("w13s", [NSL, H, 2 * I], BF16)          # per-slot [h, 2i]
    w2s = din("w2s", [NSL, I, H], BF16)                # per-slot [i, h]
    tile_e = din("tile_e", [128, TMAX], F32)           # expert id per tile
    tile_lo = din("tile_lo", [128, TMAX], F32)         # rank range lo per tile
    rhl = din("rhl", [128, NJ, 2], BF16)               # r split (r//128, r%128)
    iota42m2 = din("iota42m2", [128, TMAX, LW], F32)   # value = col - 2
    iota128r = din("iota128r", [128, TMAX, 128], BF16) # value = col (0..127)
    ident = din("ident", [128, 128], F32)
    identb = din("identb", [128, 128], BF16)
    uts128 = din("uts128", [128, 128], F32)            # strict upper: [k,m]=k<m
    rep16 = din("rep16", [16, 128], F32)               # rep16[q,p] = (p%16==q)
    sel8 = din("sel8", [128, 8, 16], F32)              # sel8[p,g,q] = (p==16g+q)
    hz = din("hz", [256, H], F32)                      # my zero-path rows
    seqidx = din("seqidx", [128, 16], I16)             # my zero-path idxs

    partial = nc.dram_tensor("partial", [T, H], F32, kind="ExternalOutput").ap()

    with tile.TileContext(nc) as tc:
        with (
            tc.tile_pool(name="const", bufs=1) as cpool,
            tc.tile_pool(name="work", bufs=2) as wpool,
            tc.tile_pool(name="persist", bufs=1) as ppool,
            tc.tile_pool(name="wload", bufs=2) as wlpool,
            tc.tile_pool(name="psum", bufs=2, space="PSUM") as pspool,
            tc.tile_pool(name="psumA", bufs=2, space="PSUM") as psapool,
            tc.tile_pool(name="dram", bufs=1, space="DRAM") as dpool,
        ):
            nc.gpsimd.load_library(library_config.mlp)

            # ---- resident constants ----
            rw_sb = cpool.tile([128, 8, NE], F32, tag="rw")
            nc.sync.dma_start(rw_sb[:], rwt.rearrange("(k p) n -> p k n", p=128))
            bias_sb = cpool.tile([128, NE], F32, tag="bias")
            nc.sync.dma_start(bias_sb[:], bias_b[:])
            ident_sb = cpool.tile([128, 128], F32, tag="ident")
            nc.sync.dma_start(ident_sb[:], ident[:])
            identb_sb = cpool.tile([128, 128], BF16, tag="identb")
            nc.sync.dma_start(identb_sb[:], identb[:])
            uts_sb = cpool.tile([128, 128], F32, tag="uts")
            nc.sync.dma_start(uts_sb[:], uts128[:])
            te_sb = cpool.tile([128, TMAX], F32, tag="te")
            nc.sync.dma_start(te_sb[:], tile_e[:])
            tlo_sb = cpool.tile([128, TMAX], F32, tag="tlo")
            nc.sync.dma_start(tlo_sb[:], tile_lo[:])
            i42_sb = cpool.tile([128, TMAX, LW], F32, tag="i42")
            nc.sync.dma_start(i42_sb[:], iota42m2[:])
            i128_sb = cpool.tile([128, TMAX, 128], BF16, tag="i128")
            nc.sync.dma_start(i128_sb[:], iota128r[:])
            rep16_sb = cpool.tile([16, 128], F32, tag="rep16")
            nc.sync.dma_start(rep16_sb[:], rep16[:])
            sel8_sb = cpool.tile([128, 8, 16], F32, tag="sel8")
            nc.sync.dma_start(sel8_sb[:], sel8[:])
            seq_sb = cpool.tile([128, 16], I16, tag="seqsb")
            nc.sync.dma_start(seq_sb[:], seqidx[:])
            zeros16 = cpool.tile([128, NJ], F32, tag="z16")
            nc.vector.memset(zeros16[:], 0.0)

            # expert one-hot per tile: oh_te_f[p, tau, 2+e] = (e == te[tau])
            oh_te_f = cpool.tile([128, TMAX, LW], F32, tag="ohtef")
            nc.vector.tensor_tensor(
                oh_te_f[:], i42_sb[:],
                te_sb[:].unsqueeze(2).to_broadcast([128, TMAX, LW]),
                op=AluOp.is_equal)
            oh_te_b = cpool.tile([128, TMAX, LW], BF16, tag="ohteb")
            nc.vector.tensor_copy(oh_te_b[:], oh_te_f[:])

            # ---- persistent intermediates ----
            lhsT_all = ppool.tile([128, NJ, LW], BF16, tag="lhsT")
            nc.sync.dma_start(lhsT_all[:, :, 0:2], rhl[:])
            sc3 = ppool.tile([128, NJ, NE], F32, tag="sc3")

            # =============== phase 1: router (exact fp32) ===============
            for cq in range(8):
                ht = wpool.tile([128, 8, 256], F32, tag="ht")
                nc.sync.dma_start(
                    ht[:],
                    hidden_T.rearrange("(k p) t -> p k t", p=128)[
                        :, :, cq * 256:(cq + 1) * 256])
                ps_lg = psapool.tile([40, 256], F32, tag="ps_big",
                                     name=f"ps_lg{cq}")
                for k in range(8):
                    nc.tensor.matmul(ps_lg[:], lhsT=rw_sb[:, k, :],
                                     rhs=ht[:, k, :],
                                     start=(k == 0), stop=(k == 7))
                lgs = wpool.tile([40, 256], F32, tag="lgs")
                nc.vector.tensor_copy(lgs[:], ps_lg[:])
                for q in range(2):
                    j = cq * 2 + q
                    ps_l = pspool.tile([128, 128], F32, tag="ps_tr",
                                       name=f"ps_lt{j}")
                    nc.tensor.transpose(ps_l[:, :NE],
                                        lgs[:, q * 128:(q + 1) * 128],
                                        ident_sb[:NE, :NE])
                    nc.vector.tensor_copy(sc3[:, j, :], ps_l[:, :NE])

            # batched softmax + top-4 mask over [128, NJ, NE]
            rmax = wpool.tile([128, NJ, 1], F32, tag="rmax")
            nc.vector.tensor_reduce(rmax[:], sc3[:], axis=AXL.X,
                                    op=AluOp.max, negate=True)  # -max
            xs = wpool.tile([128, NJ, NE], F32, tag="xs")
            nc.vector.tensor_tensor(xs[:], sc3[:],
                                    rmax[:].to_broadcast([128, NJ, NE]),
                                    op=AluOp.add)
            ex = wpool.tile([128, NJ, NE], F32, tag="ex")
            nc.scalar.activation(ex[:], xs[:], ACT_F.Exp)
            rsum = wpool.tile([128, NJ, 1], F32, tag="rsum")
            nc.vector.tensor_reduce(rsum[:], ex[:], axis=AXL.X, op=AluOp.add)
            rinv = wpool.tile([128, NJ, 1], F32, tag="rinv")
            nc.vector.reciprocal(rinv[:], rsum[:])
            scores = ppool.tile([128, NJ, NE], F32, tag="scores")
            nc.vector.tensor_tensor(scores[:], ex[:],
                                    rinv[:].to_broadcast([128, NJ, NE]),
                                    op=AluOp.mult)
            biased = ppool.tile([128, NJ, NE], F32, tag="biased")
            nc.vector.tensor_tensor(
                biased[:], scores[:],
                bias_sb[:].unsqueeze(1).to_broadcast([128, NJ, NE]),
                op=AluOp.add)
            thr = ppool.tile([128, NJ, 1], F32, tag="thr")
            for j in range(NJ):
                t8 = wpool.tile([128, 8], F32, tag="t8")
                nc.vector.max(t8[:], biased[:, j])
                nc.vector.tensor_copy(thr[:, j, :], t8[:, 3:4])
            mask3 = wpool.tile([128, NJ, NE], F32, tag="mask3")
            nc.vector.tensor_tensor(mask3[:], biased[:],
                                    thr[:].to_broadcast([128, NJ, NE]),
                                    op=AluOp.is_ge)
            mg = ppool.tile([128, NJ, NE], F32, tag="mg")
            nc.vector.scalar_tensor_tensor(mg[:], mask3[:], ROUTED_SCALE,
                                           scores[:], op0=AluOp.mult,
                                           op1=AluOp.mult)
            zt_all = ppool.tile([128, NJ], F32, tag="zt")
            nc.vector.tensor_reduce(zt_all[:], mg[:, :, ER:NE], axis=AXL.X,
                                    op=AluOp.add)
            nc.vector.tensor_copy(lhsT_all[:, :, 2:LW], mg[:])
            mask3b = ppool.tile([128, NJ, NE], BF16, tag="mask3b")
            nc.vector.tensor_copy(mask3b[:], mask3[:])

            # =============== zero-expert path (early) ===============
            zt_flat = dpool.tile([1, T], F32, tag="ztflat")
            nc.sync.dma_start(zt_flat[0, :].rearrange("(p j) -> p j", p=128),
                              zt_all[:])
            pid = nc.sync.partition_id()
            for tt in range(2):
                ztv = wpool.tile([1, 128], F32, tag="ztv")
                nc.sync.dma_start(
                    ztv[:], zt_flat[0:1, bass.ds(pid * 256 + tt * 128, 128)])
                ps_zt = psapool.tile([128, 1], F32, tag="ps_big",
                                     name=f"ps_zt{tt}")
                nc.tensor.transpose(ps_zt[:], ztv[:], ident_sb[:1, :1])
                ztc = wpool.tile([128, 1], F32, tag="ztc")
                nc.vector.tensor_copy(ztc[:], ps_zt[:])
                hzt = wpool.tile([128, H], F32, tag="hzt")
                nc.sync.dma_start(hzt[:], hz[tt * 128:(tt + 1) * 128, :])
                yz = wpool.tile([128, 1, H], F32, tag="yz")
                nc.scalar.activation(yz[:, 0, :], hzt[:], ACT_F.Copy,
                                     scale=ztc[:, 0:1])
                nc.gpsimd.dma_scatter_add(
                    out_ap=partial[:], in_ap=yz[:],
                    idxs_ap=seq_sb[:, tt * 8:(tt + 1) * 8],
                    num_idxs=128, num_idxs_reg=128, elem_size=H)

            # =============== phase 2: dispatch bookkeeping ===============
            # mask_tau[p, tau, j] = token (p,j) selected expert te[tau]
            mask_tau = ppool.tile([128, TMAX, NJ], F32, tag="masktau")
            for tau in range(TMAX):
                mtmp = wpool.tile([128, NJ, NE], BF16, tag="mtmp")
                nc.vector.tensor_tensor(
                    mtmp[:], mask3b[:],
                    oh_te_b[:, tau, 2:LW].unsqueeze(1).to_broadcast(
                        [128, NJ, NE]),
                    op=AluOp.mult)
                nc.vector.tensor_reduce(mask_tau[:, tau], mtmp[:], axis=AXL.X,
                                        op=AluOp.add)
            inrow = wpool.tile([128, TMAX, NJ], F32, tag="inrow")
            for tau in range(TMAX):
                nc.vector.tensor_tensor_scan(inrow[:, tau], mask_tau[:, tau],
                                             zeros16[:], 0.0,
                                             op0=AluOp.add, op1=AluOp.add)
            rowsum = wpool.tile([128, TMAX], F32, tag="rowsum")
            nc.vector.tensor_reduce(rowsum[:], mask_tau[:], axis=AXL.X,
                                    op=AluOp.add)
            ps_rp = psapool.tile([128, TMAX], F32, tag="ps_big",
                                  name="ps_rp")
            nc.tensor.matmul(ps_rp[:], lhsT=uts_sb[:], rhs=rowsum[:],
                             start=True, stop=True)
            pref = wpool.tile([128, TMAX, 1], F32, tag="pref")
            nc.vector.tensor_copy(pref[:, :, 0], ps_rp[:])
            pos = wpool.tile([128, TMAX, NJ], F32, tag="pos")
            nc.vector.tensor_tensor(pos[:], inrow[:],
                                    pref[:].to_broadcast([128, TMAX, NJ]),
                                    op=AluOp.add)
            nc.vector.tensor_sub(pos[:], pos[:], mask_tau[:])
            t1 = wpool.tile([128, TMAX, NJ], F32, tag="t1")
            nc.vector.tensor_tensor(
                t1[:], pos[:],
                tlo_sb[:].unsqueeze(2).to_broadcast([128, TMAX, NJ]),
                op=AluOp.subtract)
            okr = wpool.tile([128, TMAX, NJ], F32, tag="okr")
            nc.vector.tensor_scalar(okr[:], t1[:], -0.5, None, op0=AluOp.is_gt)
            ok2 = wpool.tile([128, TMAX, NJ], F32, tag="ok2")
            nc.vector.tensor_scalar(ok2[:], t1[:], 127.5, None, op0=AluOp.is_lt)
            nc.vector.tensor_mul(okr[:], okr[:], ok2[:])
            nc.vector.tensor_mul(okr[:], okr[:], mask_tau[:])
            oku = wpool.tile([128, TMAX, NJ], U8, tag="oku")
            nc.vector.tensor_copy(oku[:], okr[:])
            t1m = wpool.tile([128, TMAX, NJ], F32, tag="t1m")
            nc.vector.memset(t1m[:], -4.0)
            nc.vector.copy_predicated(t1m[:], oku[:], t1[:])
            t1mb = ppool.tile([128, TMAX, NJ], BF16, tag="t1mb")
            nc.vector.tensor_copy(t1mb[:], t1m[:])

            # one-hot inversion: inv[c, tau*128+m] = sum_tokens lhsT[t,c]*oh
            inv_ps = psapool.tile([LW, NSLOT], F32, tag="ps_big",
                                   name="ps_inv")
            for j in range(NJ):
                oh = wpool.tile([128, TMAX, 128], BF16, tag="oh")
                nc.vector.tensor_tensor(
                    oh[:], i128_sb[:],
                    t1mb[:, :, j].unsqueeze(2).to_broadcast([128, TMAX, 128]),
                    op=AluOp.is_equal)
                ohf = oh[:].rearrange("p a b -> p (a b)")
                for lo in range(0, NSLOT, 512):
                    hi = min(lo + 512, NSLOT)
                    nc.tensor.matmul(inv_ps[:, lo:hi], lhsT=lhsT_all[:, j],
                                     rhs=ohf[:, lo:hi],
                                     start=(j == 0), stop=(j == NJ - 1))

            # decode via PE transpose: tsp[p, tau, :] = inv_ps[:, tau*128+p]
            inv_sb = wpool.tile([LW, NSLOT], F32, tag="invsb")
            nc.vector.tensor_copy(inv_sb[:], inv_ps[:])
            tsp = ppool.tile([128, TMAX, LW], F32, tag="tsp")
            for tau in range(TMAX):
                ps_tsp = pspool.tile([128, 128], F32, tag="ps_tr",
                                     name=f"ps_tsp{tau}")
                nc.tensor.transpose(ps_tsp[:, :LW],
                                    inv_sb[:, tau * 128:(tau + 1) * 128],
                                    ident_sb[:LW, :LW])
                nc.vector.tensor_copy(tsp[:, tau], ps_tsp[:, :LW])
            # r = 128*hi + lo; gate = tsp . onehot_te
            r_pt = ppool.tile([128, TMAX], F32, tag="rpt")
            nc.vector.scalar_tensor_tensor(r_pt[:], tsp[:, :, 0], 128.0,
                                           tsp[:, :, 1], op0=AluOp.mult,
                                           op1=AluOp.add)
            gtmp = wpool.tile([128, TMAX, LW], F32, tag="gtmp")
            nc.vector.tensor_tensor(gtmp[:], tsp[:], oh_te_f[:], op=AluOp.mult)
            g_wr = ppool.tile([128, TMAX], F32, tag="gwr")
            nc.vector.tensor_reduce(g_wr[:], gtmp[:], axis=AXL.X, op=AluOp.add)

            # slot->token list in DGE layout, built on-chip:
            # idxw[p, tau*8+g] = r_pt[16g + p%16, tau]
            idx16 = wpool.tile([16, 8 * TMAX], F32, tag="idx16")
            for g in range(8):
                ps_g = psapool.tile([16, TMAX], F32, tag="ps_big",
                                    name=f"ps_selg{g}")
                nc.tensor.matmul(ps_g[:], lhsT=sel8_sb[:, g], rhs=r_pt[:],
                                 start=True, stop=True)
                nc.vector.tensor_copy(idx16[:, g::8], ps_g[:])
            ps_rep = psapool.tile([128, 8 * TMAX], F32, tag="ps_big",
                                   name="ps_rep")
            nc.tensor.matmul(ps_rep[:], lhsT=rep16_sb[:], rhs=idx16[:],
                             start=True, stop=True)
            idxw = ppool.tile([128, 8 * TMAX], I16, tag="idxw")
            nc.vector.tensor_copy(idxw[:], ps_rep[:])

            # =============== phase 3: gather + FFN + combine ===============
            xts = []
            for s in range(NSL):
                nt = SLOT_CAP[s]
                t0 = SLOT_TILES[s][0]
                xt = wlpool.tile([128, 8, nt * 128], BF16, tag=f"xt{nt}",
                                 name=f"xt_s{s}")
                nc.gpsimd.dma_gather(
                    out_ap=xt[:], in_ap=hidden_bf[:],
                    idxs_ap=idxw[:, t0 * 8:(t0 + nt) * 8],
                    num_idxs=nt * 128, num_idxs_reg=nt * 128, elem_size=H,
                    transpose=True)
                xts.append(xt)

            for s in range(NSL):
                nt = SLOT_CAP[s]
                xt = xts[s]
                w13_sb = wlpool.tile([128, 8, 2 * I], BF16, tag="w13")
                w2_sb = wlpool.tile([128, 4, H], BF16, tag="w2")
                nc.scalar.dma_start(w13_sb[:],
                                    w13s[s].rearrange("(k p) i -> p k i",
                                                      p=128))
                nc.scalar.dma_start(w2_sb[:],
                                    w2s[s].rearrange("(k p) i -> p k i",
                                                     p=128))
                for ti, tau in enumerate(SLOT_TILES[s]):
                    # mm1
                    ps_gu = psapool.tile([128, 2 * I], F32, tag="ps_big",
                                         name=f"ps_gu{tau}")
                    for k in range(8):
                        for n in range(2):
                            nc.tensor.matmul(
                                ps_gu[:, n * 512:(n + 1) * 512],
                                lhsT=xt[:, k, ti * 128:(ti + 1) * 128],
                                rhs=w13_sb[:, k, n * 512:(n + 1) * 512],
                                start=(k == 0), stop=(k == 7))
                    # h = silu(gate) * up
                    sg = wpool.tile([128, I], F32, tag="sg")
                    nc.scalar.activation(sg[:], ps_gu[:, :I], ACT_F.Sigmoid)
                    nc.vector.tensor_mul(sg[:], sg[:], ps_gu[:, :I])
                    hh = wpool.tile([128, I], F32, tag="hh")
                    nc.vector.tensor_mul(hh[:], sg[:], ps_gu[:, I:])
                    hT = wpool.tile([128, 4, 128], BF16, tag="hT")
                    for k in range(4):
                        ps_t2 = pspool.tile([128, 128], F32, tag="ps_tr")
                        nc.tensor.transpose(ps_t2[:],
                                            hh[:, k * 128:(k + 1) * 128],
                                            ident_sb[:])
                        nc.vector.tensor_copy(hT[:, k], ps_t2[:])
                    # mm2
                    ps_y = psapool.tile([128, H], F32, tag="ps_big",
                                        name=f"ps_y{tau}")
                    for k in range(4):
                        for n in range(2):
                            nc.tensor.matmul(
                                ps_y[:, n * 512:(n + 1) * 512],
                                lhsT=hT[:, k],
                                rhs=w2_sb[:, k, n * 512:(n + 1) * 512],
                                start=(k == 0), stop=(k == 3))
                    yv = wpool.tile([128, 1, H], F32, tag="yv")
                    nc.scalar.activation(yv[:, 0, :], ps_y[:], ACT_F.Copy,
                                         scale=g_wr[:, tau:tau + 1])
                    nc.gpsimd.dma_scatter_add(
                        out_ap=partial[:], in_ap=yv[:],
                        idxs_ap=idxw[:, tau * 8:(tau + 1) * 8],
                        num_idxs=128, num_idxs_reg=128, elem_size=H)

    nc.compile()
    _NC_CACHE[key] = nc
    return nc


# ---------------------------------------------------------------------------
# host wrapper
# ---------------------------------------------------------------------------

def make_in_maps(hidden_states, router_w, e_score_correction_bias, w13, w2):
    import ml_dtypes
    hidden_states = np.asarray(hidden_states, np.float32)
    router_w = np.asarray(router_w, np.float32)
    bias = np.asarray(e_score_correction_bias, np.float32)
    w13 = np.asarray(w13, np.float32)
    w2 = np.asarray(w2, np.float32)

    ids = _host_routing(hidden_states, router_w, bias)
    slot_expert, tiles = _schedule(ids)

    # r = (t % 128) * 16 + t // 128  <->  t = (r % 16) * 128 + r // 16
    r_of_t = (np.arange(T) % 128) * 16 + np.arange(T) // 128
    t_of_r = np.empty(T, np.int64)
    t_of_r[r_of_t] = np.arange(T)

    hidden_T = np.ascontiguousarray(hidden_states.T)
    hidden_rows = np.ascontiguousarray(hidden_states[t_of_r])
    hidden_bf = hidden_rows.astype(ml_dtypes.bfloat16)
    rwt = np.ascontiguousarray(router_w.T)
    bias_b = np.tile(bias[None, :], (128, 1))
    w13t = np.ascontiguousarray(w13.transpose(0, 2, 1))   # [e, h, 2I]
    w2t = np.ascontiguousarray(w2.transpose(0, 2, 1))     # [e, i, h]

    rr = np.arange(T).reshape(128, NJ).astype(np.float32)  # r at [p, j]
    rhl = np.stack([rr // 128, rr % 128], axis=-1).astype(ml_dtypes.bfloat16)
    iota42m2 = np.tile(np.arange(-2, NE, dtype=np.float32), (128, TMAX, 1))
    iota128r = np.tile(np.arange(128, dtype=np.float32), (128, TMAX, 1)) \
        .astype(ml_dtypes.bfloat16)
    ident = np.eye(128, dtype=np.float32)
    identb = np.eye(128, dtype=ml_dtypes.bfloat16)
    uts128 = np.triu(np.ones((128, 128), np.float32), k=1)
    rep16 = np.zeros((16, 128), np.float32)
    rep16[np.arange(128) % 16, np.arange(128)] = 1.0
    sel8 = np.zeros((128, 8, 16), np.float32)
    for g in range(8):
        sel8[16 * g + np.arange(16), g, np.arange(16)] = 1.0

    p_ = np.arange(128)[:, None]
    f_ = np.arange(16)[None, :]
    seq_base = (f_ % 8) * 16 + (p_ % 16) + (f_ // 8) * 128  # [p, f]

    in_maps = []
    for c in range(NCORES):
        te = np.array([tiles[c][tau][0] for tau in range(TMAX)], np.float32)
        tlo = np.array([tiles[c][tau][1] for tau in range(TMAX)], np.float32)
        in_maps.append({
            "hidden_T": hidden_T,
            "hidden_bf": hidden_bf,
            "rwt": rwt,
            "bias_b": bias_b,
            "w13s": np.ascontiguousarray(
                w13t[[slot_expert[c][s] for s in range(NSL)]]).astype(
                    ml_dtypes.bfloat16),
            "w2s": np.ascontiguousarray(
                w2t[[slot_expert[c][s] for s in range(NSL)]]).astype(
                    ml_dtypes.bfloat16),
            "tile_e": np.tile(te[None, :], (128, 1)),
            "tile_lo": np.tile(tlo[None, :], (128, 1)),
            "rhl": rhl,
            "iota42m2": iota42m2,
            "iota128r": iota128r,
            "ident": ident,
            "identb": identb,
            "uts128": uts128,
            "rep16": rep16,
            "sel8": sel8,
            "hz": np.ascontiguousarray(hidden_rows[c * 256:(c + 1) * 256]),
            "seqidx": (seq_base + c * 256).astype(np.int16),
        })
    return in_maps, t_of_r


def kernel(hidden_states, router_w, e_score_correction_bias, w13, w2,
           _trace=False):
    nc = build_nc()
    in_maps, t_of_r = make_in_maps(hidden_states, router_w,
                                   e_score_correction_bias, w13, w2)
    res = run_bass_kernel_spmd(nc, in_maps, core_ids=list(range(NCORES)),
                               trace=_trace)
    total = np.zeros((T, H), np.float64)
    for c in range(NCORES):
        total += res.results[c]["partial"].astype(np.float64)
    out = np.empty((T, H), np.float32)
    out[t_of_r] = total.astype(np.float32)      # out[t] = total[r(t)]
    kernel._last_results = res
    return out


# revision 6
# speedup vs baseline: 1.5014x; 1.2053x over previous
"""LongcatFlash MoE kernel for 8 TRN2 NeuronCores (expert-parallel).

Contract: kernel(**inputs) takes the FULL un-sharded inputs from
reference.setup_inputs() and returns the FULL [T, H] output.

Strategy (expert-parallel, memory-regime), v3:
  - Router runs replicated on every core in exact fp32; softmax/top-4
    threshold work is batched as 3D vector ops, chunked per 512 tokens
    so it overlaps the router matmul stream.
  - Selection is a mask (biased >= 4th-largest); gates flow as the
    masked-scaled score vector (40 wide) straight into the dispatch
    inversion matmul (42-wide lhsT), so no top-k extraction is needed.
  - Experts are sharded across cores by a host-computed static tile
    schedule (load-balancing metadata only; routing is on device).
  - Dispatch: rank prefix-sums (scan + triangular matmul), one bf16
    one-hot is_eq per token tile, inversion matmuls, PE-transpose
    decode; the slot->token list is built on-chip with permutation
    matmuls (no DRAM round-trip).
  - FFN: bf16 weights DMA'd directly in host-tiled contiguous layout;
    token rows gathered pre-transposed via dma_gather(transpose=True)
    from a bf16 copy of hidden. Only 4 h-transposes per tile on PE.
    Output rows accumulate per weight-slot and are scatter-added once
    per slot.
  - Zero-experts (ids >= 32) reduce to a per-token scale of the hidden
    row, applied by the token-range owner core.
  - Host unshards by summing the 8 partial outputs and undoing the row
    permutation r = (t % 128) * 16 + t // 128.
"""

import numpy as np

import concourse.bacc as bacc
import concourse.bass as bass
import concourse.mybir as mybir
import concourse.tile as tile
from concourse import library_config
from concourse.bass_utils import run_bass_kernel_spmd

F32 = mybir.dt.float32
BF16 = mybir.dt.bfloat16
I16 = mybir.dt.int16
U8 = mybir.dt.uint8

T, H, I = 2048, 1024, 512
NE, ER = 40, 32
ROUTED_SCALE = 2.5
NCORES = 8
NJ = T // 128              # 16 token tiles (r = p*16 + j)
TMAX = 9                   # static FFN tiles per core
NSL = 5                    # weight slots per core
SLOT_CAP = [4, 2, 1, 1, 1]
SLOT_TILES = [[0, 1, 2, 3], [4, 5], [6], [7], [8]]
NSLOT = TMAX * 128         # 1152 dispatch slots per core
LW = 2 + NE                # inversion lhsT width: r_hi, r_lo, 40 gates
AluOp = mybir.AluOpType
ACT_F = mybir.ActivationFunctionType
AXL = mybir.AxisListType


# ---------------------------------------------------------------------------
# host-side schedule
# ---------------------------------------------------------------------------

def _host_routing(hidden, router_w, bias):
    """fp32 routing on host — used ONLY for load-balance scheduling."""
    logits = hidden.astype(np.float32) @ router_w.astype(np.float32).T
    m = logits.max(axis=1, keepdims=True)
    e = np.exp(logits - m)
    scores = e / e.sum(axis=1, keepdims=True)
    biased = scores + bias[None, :]
    ids = np.argsort(-biased, axis=1, kind="stable")[:, :4]
    return ids


def _schedule(ids):
    """Static tile schedule: split-anywhere first-fit-decreasing packing.

    Returns per-core:
      slot_expert[c][s]: global expert id serviced by local weight slot s
      tiles[c][tau]: (expert_id, lo_rank) — dispatch range for FFN tile tau
    Ranks are positions within an expert's selected-token list in r-order.
    """
    counts = np.zeros(ER, np.int64)
    for row in ids:
        for e in row:
            if e < ER:
                counts[e] += 1
    pieces = [[e, 0, (int(counts[e]) + 127) // 128] for e in range(ER)
              if counts[e] > 0]               # [expert, first_tile, ntiles]
    pieces.sort(key=lambda p: -p[2])
    slots = sorted(((SLOT_CAP[s], c, s) for c in range(NCORES)
                    for s in range(NSL)), key=lambda x: -x[0])
    slot_expert = [[0] * NSL for _ in range(NCORES)]
    tiles = [[(0, 1 << 14)] * TMAX for _ in range(NCORES)]
    si = 0
    work = []
    for p in pieces:
        work.append(p)
    while work:
        work.sort(key=lambda p: -p[2])
        p = work.pop(0)
        if si >= len(slots):
            raise RuntimeError("schedule: out of weight slots")
        cap, c, s = slots[si]
        si += 1
        take = min(cap, p[2])
        slot_expert[c][s] = p[0]
        for k in range(cap):
            tau = SLOT_TILES[s][k]
            # tiles beyond `take` extend the range as harmless slack
            tiles[c][tau] = (p[0], 128 * (p[1] + min(k, take)))
        for k in range(take):
            tiles[c][SLOT_TILES[s][k]] = (p[0], 128 * (p[1] + k))
        if p[2] > take:
            work.append([p[0], p[1] + take, p[2] - take])
    return slot_expert, tiles


# ---------------------------------------------------------------------------
# device graph
# ---------------------------------------------------------------------------

_NC_CACHE = {}


def build_nc():
    key = "v3"
    if key in _NC_CACHE:
        return _NC_CACHE[key]
    nc = bacc.Bacc("TRN2", target_bir_lowering=False, debug=False,
                   num_devices=NCORES)

    def din(name, shape, dt):
        return nc.dram_tensor(name, shape, dt, kind="ExternalInput").ap()

    hidden_Tt = din("hidden_Tt", [8, 128, 8, 256], F32)  # router chunks
    hidden_bf = din("hidden_bf", [T, H], BF16)         # r-ordered rows, bf16
    rwt = din("rwt", [H, NE], F32)                     # router_w.T
    bias_b = din("bias_b", [128, NE], F32)             # bias replicated
    w13s = din("w13s", [NSL, 128, 8, 2 * I], BF16)     # [slot, p, k, 2i]
    w2s = din("w2s", [NSL, 128, 4, H], BF16)           # [slot, p, k, h]
    tile_e = din("tile_e", [128, TMAX], F32)           # expert id per tile
    tile_lo = din("tile_lo", [128, TMAX], F32)         # rank range lo per tile
    rhl = din("rhl", [128, NJ, 2], BF16)               # r split (r//128, r%128)
    iota42m2 = din("iota42m2", [128, TMAX, LW], F32)   # value = col - 2
    iota128r = din("iota128r", [128, TMAX, 128], BF16) # value = col (0..127)
    ident = din("ident", [128, 128], F32)
    uts128 = din("uts128", [128, 128], F32)            # strict upper: [k,m]=k<m
    rep16 = din("rep16", [16, 128], F32)               # rep16[q,p] = (p%16==q)
    sel8 = din("sel8", [128, 8, 16], F32)              # sel8[p,g,q] = (p==16g+q)
    hz = din("hz", [256, H], F32)                      # my zero-path rows
    seqidx = din("seqidx", [128, 16], I16)             # my zero-path idxs

    partial = nc.dram_tensor("partial", [T, H], F32, kind="ExternalOutput").ap()

    with tile.TileContext(nc) as tc:
        with (
            tc.tile_pool(name="const", bufs=1) as cpool,
            tc.tile_pool(name="work", bufs=2) as wpool,
            tc.tile_pool(name="persist", bufs=1) as ppool,
            tc.tile_pool(name="wload", bufs=2) as wlpool,
            tc.tile_pool(name="psum", bufs=2, space="PSUM") as pspool,
            tc.tile_pool(name="psumA", bufs=2, space="PSUM") as psapool,
            tc.tile_pool(name="dram", bufs=1, space="DRAM") as dpool,
        ):
            nc.gpsimd.load_library(library_config.mlp)

            # ---- router weights + token chunks first on the sync queue ----
            rw_sb = cpool.tile([128, 8, NE], F32, tag="rw")
            nc.sync.dma_start(rw_sb[:],
                              rwt.rearrange("(k p) n -> p k n", p=128))
            ident_sb = cpool.tile([128, 128], F32, tag="ident")
            nc.sync.dma_start(ident_sb[:], ident[:])
            hts = []
            for cq in range(8):
                ht = wlpool.tile([128, 8, 256], F32, tag="ht",
                                 name=f"ht{cq}")
                nc.sync.dma_start(ht[:], hidden_Tt[cq])
                hts.append(ht)

            # ---- resident constants (gpsimd DMA queue) ----
            bias_sb = cpool.tile([128, NE], F32, tag="bias")
            nc.gpsimd.dma_start(bias_sb[:], bias_b[:])
            uts_sb = cpool.tile([128, 128], F32, tag="uts")
            nc.gpsimd.dma_start(uts_sb[:], uts128[:])
            te_sb = cpool.tile([128, TMAX], F32, tag="te")
            nc.gpsimd.dma_start(te_sb[:], tile_e[:])
            tlo_sb = cpool.tile([128, TMAX], F32, tag="tlo")
            nc.gpsimd.dma_start(tlo_sb[:], tile_lo[:])
            i42_sb = cpool.tile([128, TMAX, LW], F32, tag="i42")
            nc.gpsimd.dma_start(i42_sb[:], iota42m2[:])
            i128_sb = cpool.tile([128, TMAX, 128], BF16, tag="i128")
            nc.gpsimd.dma_start(i128_sb[:], iota128r[:])
            rep16_sb = cpool.tile([16, 128], F32, tag="rep16")
            nc.gpsimd.dma_start(rep16_sb[:], rep16[:])
            sel8_sb = cpool.tile([128, 8, 16], F32, tag="sel8")
            nc.gpsimd.dma_start(sel8_sb[:], sel8[:])
            seq_sb = cpool.tile([128, 16], I16, tag="seqsb")
            nc.gpsimd.dma_start(seq_sb[:], seqidx[:])
            zeros16 = cpool.tile([128, NJ], F32, tag="z16")
            nc.vector.memset(zeros16[:], 0.0)

            # expert one-hot per tile: oh_te_f[p, tau, 2+e] = (e == te[tau])
            oh_te_f = cpool.tile([128, TMAX, LW], F32, tag="ohtef")
            nc.vector.tensor_tensor(
                oh_te_f[:], i42_sb[:],
                te_sb[:].unsqueeze(2).to_broadcast([128, TMAX, LW]),
                op=AluOp.is_equal)
            oh_te_b = cpool.tile([128, TMAX, LW], BF16, tag="ohteb")
            nc.vector.tensor_copy(oh_te_b[:], oh_te_f[:])

            # ---- persistent intermediates ----
            lhsT_all = ppool.tile([128, NJ, LW], BF16, tag="lhsT")
            nc.gpsimd.dma_start(lhsT_all[:, :, 0:2], rhl[:])
            sc3 = ppool.tile([128, NJ, NE], F32, tag="sc3")
            scores = ppool.tile([128, NJ, NE], F32, tag="scores")
            biased = ppool.tile([128, NJ, NE], F32, tag="biased")
            thr = ppool.tile([128, NJ, 1], F32, tag="thr")

            # =============== phase 1: router (exact fp32) ===============
            # chunked: matmul cq, transpose to token-major, then after each
            # odd cq run the softmax chain on that 4-j group (overlaps PE).
            for cq in range(8):
                ht = hts[cq]
                ps_lg = psapool.tile([40, 256], F32, tag="ps_big",
                                     name=f"ps_lg{cq}")
                for k in range(8):
                    nc.tensor.matmul(ps_lg[:], lhsT=rw_sb[:, k, :],
                                     rhs=ht[:, k, :],
                                     start=(k == 0), stop=(k == 7))
                lgs = wpool.tile([40, 256], F32, tag="lgs")
                nc.vector.tensor_copy(lgs[:], ps_lg[:])
                for q in range(2):
                    j = cq * 2 + q
                    ps_l = pspool.tile([128, 128], F32, tag="ps_tr",
                                       name=f"ps_lt{j}")
                    nc.tensor.transpose(ps_l[:, :NE],
                                        lgs[:, q * 128:(q + 1) * 128],
                                        ident_sb[:NE, :NE])
                    nc.vector.tensor_copy(sc3[:, j, :], ps_l[:, :NE])
                if cq % 2 == 0:
                    continue
                g = cq // 2
                js = slice(g * 4, g * 4 + 4)
                sh = [128, 4, NE]
                rmax = wpool.tile([128, 4, 1], F32, tag="rmax")
                nc.vector.tensor_reduce(rmax[:], sc3[:, js], axis=AXL.X,
                                        op=AluOp.max, negate=True)  # -max
                xs = wpool.tile(sh, F32, tag="xs")
                nc.vector.tensor_tensor(xs[:], sc3[:, js],
                                        rmax[:].to_broadcast(sh),
                                        op=AluOp.add)
                exv = wpool.tile(sh, F32, tag="ex")
                nc.scalar.activation(exv[:], xs[:], ACT_F.Exp)
                rsum = wpool.tile([128, 4, 1], F32, tag="rsum")
                nc.vector.tensor_reduce(rsum[:], exv[:], axis=AXL.X,
                                        op=AluOp.add)
                rinv = wpool.tile([128, 4, 1], F32, tag="rinv")
                nc.vector.reciprocal(rinv[:], rsum[:])
                nc.vector.tensor_tensor(scores[:, js], exv[:],
                                        rinv[:].to_broadcast(sh),
                                        op=AluOp.mult)
                nc.vector.tensor_tensor(
                    biased[:, js], scores[:, js],
                    bias_sb[:].unsqueeze(1).to_broadcast(sh),
                    op=AluOp.add)
                for j in range(g * 4, g * 4 + 4):
                    t8 = wpool.tile([128, 8], F32, tag="t8")
                    nc.vector.max(t8[:], biased[:, j])
                    nc.vector.tensor_copy(thr[:, j, :], t8[:, 3:4])

            mask3 = wpool.tile([128, NJ, NE], F32, tag="mask3")
            nc.vector.tensor_tensor(mask3[:], biased[:],
                                    thr[:].to_broadcast([128, NJ, NE]),
                                    op=AluOp.is_ge)
            mg = ppool.tile([128, NJ, NE], F32, tag="mg")
            nc.vector.scalar_tensor_tensor(mg[:], mask3[:], ROUTED_SCALE,
                                           scores[:], op0=AluOp.mult,
                                           op1=AluOp.mult)
            zt_all = ppool.tile([128, NJ], F32, tag="zt")
            nc.vector.tensor_reduce(zt_all[:], mg[:, :, ER:NE], axis=AXL.X,
                                    op=AluOp.add)
            nc.vector.tensor_copy(lhsT_all[:, :, 2:LW], mg[:])
            mask3b = ppool.tile([128, NJ, NE], BF16, tag="mask3b")
            nc.vector.tensor_copy(mask3b[:], mask3[:])

            # =============== zero-expert path (early) ===============
            zt_flat = dpool.tile([1, T], F32, tag="ztflat")
            nc.sync.dma_start(zt_flat[0, :].rearrange("(p j) -> p j", p=128),
                              zt_all[:])
            pid = nc.sync.partition_id()
            yz = ppool.tile([128, 2, H], F32, tag="yz")
            for tt in range(2):
                ztv = wpool.tile([1, 128], F32, tag="ztv")
                nc.sync.dma_start(
                    ztv[:], zt_flat[0:1, bass.ds(pid * 256 + tt * 128, 128)])
                ps_zt = psapool.tile([128, 1], F32, tag="ps_big",
                                     name=f"ps_zt{tt}")
                nc.tensor.transpose(ps_zt[:], ztv[:], ident_sb[:1, :1])
                ztc = wpool.tile([128, 1], F32, tag="ztc")
                nc.vector.tensor_copy(ztc[:], ps_zt[:])
                hzt = wpool.tile([128, H], F32, tag="hzt")
                nc.sync.dma_start(hzt[:], hz[tt * 128:(tt + 1) * 128, :])
                nc.scalar.activation(yz[:, tt, :], hzt[:], ACT_F.Copy,
                                     scale=ztc[:, 0:1])
            nc.gpsimd.dma_scatter_add(
                out_ap=partial[:], in_ap=yz[:], idxs_ap=seq_sb[:],
                num_idxs=256, num_idxs_reg=256, elem_size=H)

            # =============== phase 2: dispatch bookkeeping ===============
            # mask_tau[p, tau, j] = token (p,j) selected expert te[tau]
            mask_tau = ppool.tile([128, TMAX, NJ], F32, tag="masktau")
            for tau in range(TMAX):
                mtmp = wpool.tile([128, NJ, NE], BF16, tag="mtmp")
                nc.vector.tensor_tensor(
                    mtmp[:], mask3b[:],
                    oh_te_b[:, tau, 2:LW].unsqueeze(1).to_broadcast(
                        [128, NJ, NE]),
                    op=AluOp.mult)
                nc.vector.tensor_reduce(mask_tau[:, tau], mtmp[:], axis=AXL.X,
                                        op=AluOp.add)
            inrow = wpool.tile([128, TMAX, NJ], F32, tag="inrow")
            for tau in range(TMAX):
                nc.vector.tensor_tensor_scan(inrow[:, tau], mask_tau[:, tau],
                                             zeros16[:], 0.0,
                                             op0=AluOp.add, op1=AluOp.add)
            rowsum = wpool.tile([128, TMAX], F32, tag="rowsum")
            nc.vector.tensor_reduce(rowsum[:], mask_tau[:], axis=AXL.X,
                                    op=AluOp.add)
            ps_rp = psapool.tile([128, TMAX], F32, tag="ps_big",
                                 name="ps_rp")
            nc.tensor.matmul(ps_rp[:], lhsT=uts_sb[:], rhs=rowsum[:],
                             start=True, stop=True)
            pref = wpool.tile([128, TMAX, 1], F32, tag="pref")
            nc.vector.tensor_copy(pref[:, :, 0], ps_rp[:])
            pos = wpool.tile([128, TMAX, NJ], F32, tag="pos")
            nc.vector.tensor_tensor(pos[:], inrow[:],
                                    pref[:].to_broadcast([128, TMAX, NJ]),
                                    op=AluOp.add)
            nc.vector.tensor_sub(pos[:], pos[:], mask_tau[:])
            t1 = wpool.tile([128, TMAX, NJ], F32, tag="t1")
            nc.vector.tensor_tensor(
                t1[:], pos[:],
                tlo_sb[:].unsqueeze(2).to_broadcast([128, TMAX, NJ]),
                op=AluOp.subtract)
            okr = wpool.tile([128, TMAX, NJ], F32, tag="okr")
            nc.vector.tensor_scalar(okr[:], t1[:], -0.5, None, op0=AluOp.is_gt)
            ok2 = wpool.tile([128, TMAX, NJ], F32, tag="ok2")
            nc.vector.tensor_scalar(ok2[:], t1[:], 127.5, None, op0=AluOp.is_lt)
            nc.vector.tensor_mul(okr[:], okr[:], ok2[:])
            nc.vector.tensor_mul(okr[:], okr[:], mask_tau[:])
            oku = wpool.tile([128, TMAX, NJ], U8, tag="oku")
            nc.vector.tensor_copy(oku[:], okr[:])
            t1m = wpool.tile([128, TMAX, NJ], F32, tag="t1m")
            nc.vector.memset(t1m[:], -4.0)
            nc.vector.copy_predicated(t1m[:], oku[:], t1[:])
            t1mb = ppool.tile([128, TMAX, NJ], BF16, tag="t1mb")
            nc.vector.tensor_copy(t1mb[:], t1m[:])

            # one-hot inversion: inv[c, tau*128+m] = sum_tokens lhsT[t,c]*oh
            inv_ps = psapool.tile([LW, NSLOT], F32, tag="ps_big",
                                  name="ps_inv")
            for j in range(NJ):
                oh = wpool.tile([128, TMAX, 128], BF16, tag="oh")
                nc.vector.tensor_tensor(
                    oh[:],
                    t1mb[:, :, j].unsqueeze(2).to_broadcast([128, TMAX, 128]),
                    i128_sb[:],
                    op=AluOp.is_equal)
                ohf = oh[:].rearrange("p a b -> p (a b)")
                for lo in range(0, NSLOT, 512):
                    hi = min(lo + 512, NSLOT)
                    nc.tensor.matmul(inv_ps[:, lo:hi], lhsT=lhsT_all[:, j],
                                     rhs=ohf[:, lo:hi],
                                     start=(j == 0), stop=(j == NJ - 1))

            # decode via PE transpose: tsp[p, tau, :] = inv_ps[:, tau*128+p]
            inv_sb = ppool.tile([LW, NSLOT], F32, tag="invsb")
            nc.vector.tensor_copy(inv_sb[:], inv_ps[:])
            tsp = ppool.tile([128, TMAX, LW], F32, tag="tsp")
            for tau in range(TMAX):
                ps_tsp = pspool.tile([128, 128], F32, tag="ps_tr",
                                     name=f"ps_tsp{tau}")
                nc.tensor.transpose(ps_tsp[:, :LW],
                                    inv_sb[:, tau * 128:(tau + 1) * 128],
                                    ident_sb[:LW, :LW])
                nc.vector.tensor_copy(tsp[:, tau], ps_tsp[:, :LW])
            # r = 128*hi + lo; gate = tsp . onehot_te
            r_pt = ppool.tile([128, TMAX], F32, tag="rpt")
            nc.vector.scalar_tensor_tensor(r_pt[:], tsp[:, :, 0], 128.0,
                                           tsp[:, :, 1], op0=AluOp.mult,
                                           op1=AluOp.add)
            gtmp = wpool.tile([128, TMAX, LW], F32, tag="gtmp")
            nc.vector.tensor_tensor(gtmp[:], tsp[:], oh_te_f[:], op=AluOp.mult)
            g_wr = ppool.tile([128, TMAX], F32, tag="gwr")
            nc.vector.tensor_reduce(g_wr[:], gtmp[:], axis=AXL.X, op=AluOp.add)

            # slot->token list in DGE layout, built on-chip:
            # idxw[p, tau*8+g] = r_pt[16g + p%16, tau]
            idx16 = wpool.tile([16, 8 * TMAX], F32, tag="idx16")
            for g in range(8):
                ps_g = psapool.tile([16, TMAX], F32, tag="ps_big",
                                    name=f"ps_selg{g}")
                nc.tensor.matmul(ps_g[:], lhsT=sel8_sb[:, g], rhs=r_pt[:],
                                 start=True, stop=True)
                nc.vector.tensor_copy(idx16[:, g::8], ps_g[:])
            ps_rep = psapool.tile([128, 8 * TMAX], F32, tag="ps_big",
                                  name="ps_rep")
            nc.tensor.matmul(ps_rep[:], lhsT=rep16_sb[:], rhs=idx16[:],
                             start=True, stop=True)
            idxw = ppool.tile([128, 8 * TMAX], I16, tag="idxw")
            nc.vector.tensor_copy(idxw[:], ps_rep[:])

            # =============== phase 3: gather + FFN + combine ===============
            xts = []
            for s in range(NSL):
                nt = SLOT_CAP[s]
                t0 = SLOT_TILES[s][0]
                xt = wlpool.tile([128, 8, nt * 128], BF16, tag=f"xt{nt}",
                                 name=f"xt_s{s}")
                nc.gpsimd.dma_gather(
                    out_ap=xt[:], in_ap=hidden_bf[:],
                    idxs_ap=idxw[:, t0 * 8:(t0 + nt) * 8],
                    num_idxs=nt * 128, num_idxs_reg=nt * 128, elem_size=H,
                    transpose=True)
                xts.append(xt)

            for s in range(NSL):
                nt = SLOT_CAP[s]
                t0 = SLOT_TILES[s][0]
                xt = xts[s]
                w13_sb = wlpool.tile([128, 8, 2 * I], BF16, tag="w13")
                w2_sb = wlpool.tile([128, 4, H], BF16, tag="w2")
                nc.scalar.dma_start(w13_sb[:], w13s[s])
                nc.scalar.dma_start(w2_sb[:], w2s[s])
                groups = [SLOT_TILES[s][i:i + 2]
                          for i in range(0, nt, 2)]
                for gi, grp in enumerate(groups):
                    gn = len(grp)
                    gt0 = grp[0]
                    yv = wpool.tile([128, gn, H], F32, tag=f"yv{gn}",
                                    name=f"yv_s{s}g{gi}")
                    for ti, tau in enumerate(grp):
                        # mm1
                        xti = SLOT_TILES[s].index(tau)
                        ps_gu = psapool.tile([128, 2 * I], F32, tag="ps_big",
                                             name=f"ps_gu{tau}")
                        for k in range(8):
                            for n in range(2):
                                nc.tensor.matmul(
                                    ps_gu[:, n * 512:(n + 1) * 512],
                                    lhsT=xt[:, k, xti * 128:(xti + 1) * 128],
                                    rhs=w13_sb[:, k, n * 512:(n + 1) * 512],
                                    start=(k == 0), stop=(k == 7))
                        # h = silu(gate) * up
                        sg = wpool.tile([128, I], F32, tag="sg")
                        nc.scalar.activation(sg[:], ps_gu[:, :I],
                                             ACT_F.Sigmoid)
                        nc.vector.tensor_mul(sg[:], sg[:], ps_gu[:, :I])
                        hh = wpool.tile([128, I], F32, tag="hh")
                        nc.vector.tensor_mul(hh[:], sg[:], ps_gu[:, I:])
                        hT = wpool.tile([128, 4, 128], BF16, tag="hT")
                        for k in range(4):
                            ps_t2 = pspool.tile([128, 128], F32, tag="ps_tr")
                            nc.tensor.transpose(ps_t2[:],
                                                hh[:, k * 128:(k + 1) * 128],
                                                ident_sb[:])
                            nc.vector.tensor_copy(hT[:, k], ps_t2[:])
                        # mm2
                        ps_y = psapool.tile([128, H], F32, tag="ps_big",
                                            name=f"ps_y{tau}")
                        for k in range(4):
                            for n in range(2):
                                nc.tensor.matmul(
                                    ps_y[:, n * 512:(n + 1) * 512],
                                    lhsT=hT[:, k],
                                    rhs=w2_sb[:, k, n * 512:(n + 1) * 512],
                                    start=(k == 0), stop=(k == 3))
                        nc.scalar.activation(yv[:, ti, :], ps_y[:],
                                             ACT_F.Copy,
                                             scale=g_wr[:, tau:tau + 1])
                    nc.gpsimd.dma_scatter_add(
                        out_ap=partial[:], in_ap=yv[:],
                        idxs_ap=idxw[:, gt0 * 8:(gt0 + gn) * 8],
                        num_idxs=gn * 128, num_idxs_reg=gn * 128,
                        elem_size=H)

    nc.compile()
    _NC_CACHE[key] = nc
    return nc


# ---------------------------------------------------------------------------
# host wrapper
# ---------------------------------------------------------------------------

def make_in_maps(hidden_states, router_w, e_score_correction_bias, w13, w2):
    import ml_dtypes
    hidden_states = np.asarray(hidden_states, np.float32)
    router_w = np.asarray(router_w, np.float32)
    bias = np.asarray(e_score_correction_bias, np.float32)
    w13 = np.asarray(w13, np.float32)
    w2 = np.asarray(w2, np.float32)

    ids = _host_routing(hidden_states, router_w, bias)
    slot_expert, tiles = _schedule(ids)

    # r = (t % 128) * 16 + t // 128  <->  t = (r % 16) * 128 + r // 16
    r_of_t = (np.arange(T) % 128) * 16 + np.arange(T) // 128
    t_of_r = np.empty(T, np.int64)
    t_of_r[r_of_t] = np.arange(T)

    hidden_T = np.ascontiguousarray(hidden_states.T)
    # [k, p, cq, u] -> [cq, p, k, u]
    hidden_Tt = np.ascontiguousarray(
        hidden_T.reshape(8, 128, 8, 256).transpose(2, 1, 0, 3))
    hidden_rows = np.ascontiguousarray(hidden_states[t_of_r])
    hidden_bf = hidden_rows.astype(ml_dtypes.bfloat16)
    rwt = np.ascontiguousarray(router_w.T)
    bias_b = np.tile(bias[None, :], (128, 1))
    w13t = w13.transpose(0, 2, 1)                  # [e, h, 2I]
    w2t = w2.transpose(0, 2, 1)                    # [e, i, h]
    # host-tiled contiguous weight layout: [e, p, k, i]
    w13tt = np.ascontiguousarray(
        w13t.reshape(ER, 8, 128, 2 * I).transpose(0, 2, 1, 3)).astype(
            ml_dtypes.bfloat16)
    w2tt = np.ascontiguousarray(
        w2t.reshape(ER, 4, 128, H).transpose(0, 2, 1, 3)).astype(
            ml_dtypes.bfloat16)

    rr = np.arange(T).reshape(128, NJ).astype(np.float32)  # r at [p, j]
    rhl = np.stack([rr // 128, rr % 128], axis=-1).astype(ml_dtypes.bfloat16)
    iota42m2 = np.tile(np.arange(-2, NE, dtype=np.float32), (128, TMAX, 1))
    iota128r = np.tile(np.arange(128, dtype=np.float32), (128, TMAX, 1)) \
        .astype(ml_dtypes.bfloat16)
    ident = np.eye(128, dtype=np.float32)
    uts128 = np.triu(np.ones((128, 128), np.float32), k=1)
    rep16 = np.zeros((16, 128), np.float32)
    rep16[np.arange(128) % 16, np.arange(128)] = 1.0
    sel8 = np.zeros((128, 8, 16), np.float32)
    for g in range(8):
        sel8[16 * g + np.arange(16), g, np.arange(16)] = 1.0

    p_ = np.arange(128)[:, None]
    f_ = np.arange(16)[None, :]
    seq_base = (f_ % 8) * 16 + (p_ % 16) + (f_ // 8) * 128  # [p, f]

    in_maps = []
    for c in range(NCORES):
        te = np.array([tiles[c][tau][0] for tau in range(TMAX)], np.float32)
        tlo = np.array([tiles[c][tau][1] for tau in range(TMAX)], np.float32)
        in_maps.append({
            "hidden_Tt": hidden_Tt,
            "hidden_bf": hidden_bf,
            "rwt": rwt,
            "bias_b": bias_b,
            "w13s": np.ascontiguousarray(
                w13tt[[slot_expert[c][s] for s in range(NSL)]]),
            "w2s": np.ascontiguousarray(
                w2tt[[slot_expert[c][s] for s in range(NSL)]]),
            "tile_e": np.tile(te[None, :], (128, 1)),
            "tile_lo": np.tile(tlo[None, :], (128, 1)),
            "rhl": rhl,
            "iota42m2": iota42m2,
            "iota128r": iota128r,
            "ident": ident,
            "uts128": uts128,
            "rep16": rep16,
            "sel8": sel8,
            "hz": np.ascontiguousarray(hidden_rows[c * 256:(c + 1) * 256]),
            "seqidx": (seq_base + c * 256).astype(np.int16),
        })
    return in_maps, t_of_r


def kernel(hidden_states, router_w, e_score_correction_bias, w13, w2,
           _trace=False):
    nc = build_nc()
    in_maps, t_of_r = make_in_maps(hidden_states, router_w,
                                   e_score_correction_bias, w13, w2)
    res = run_bass_kernel_spmd(nc, in_maps, core_ids=list(range(NCORES)),
                               trace=_trace)
    total = np.zeros((T, H), np.float64)
    for c in range(NCORES):
        total += res.results[c]["partial"].astype(np.float64)
    out = np.empty((T, H), np.float32)
    out[t_of_r] = total.astype(np.float32)      # out[t] = total[r(t)]
    kernel._last_results = res
    return out


# revision 9
# speedup vs baseline: 1.5021x; 1.0005x over previous
"""LongcatFlash MoE kernel for 8 TRN2 NeuronCores (expert-parallel).

Contract: kernel(**inputs) takes the FULL un-sharded inputs from
reference.setup_inputs() and returns the FULL [T, H] output.

Strategy (expert-parallel, memory-regime), v4:
  - Router replicated, exact fp32 matmuls; PE pre-warmed with dummy
    matmuls so the stream runs at full clock. Softmax / top-4-threshold
    / mask / per-tile-membership vector work is chunked per 512 tokens
    and runs under the router matmul stream.
  - Selection is a mask (biased >= 4th-largest); gates flow as the
    masked-scaled score vector straight into the inversion matmul.
  - Dispatch inversion is split into two tau-groups: slot-0's tiles
    decode + gather + compute while the second group still inverts.
  - Slot->token lists built on-chip with permutation matmuls.
  - FFN: bf16 weights (DMA gated behind router start so the hidden
    chunks win early bandwidth), transposed bf16 gather for x, Silu on
    the scalar engine, bf16 partial output with CCE scatter-add.
  - Zero-experts (ids >= 32) reduce to a per-token scale of the hidden
    row, applied by the token-range owner core.
  - Host unshards by summing the 8 partial outputs and undoing the row
    permutation r = (t % 128) * 16 + t // 128.
"""

import numpy as np

import concourse.bacc as bacc
import concourse.bass as bass
import concourse.mybir as mybir
import concourse.tile as tile
from concourse import library_config
from concourse.bass_utils import run_bass_kernel_spmd

F32 = mybir.dt.float32
BF16 = mybir.dt.bfloat16
I16 = mybir.dt.int16
U8 = mybir.dt.uint8

T, H, I = 2048, 1024, 512
NE, ER = 40, 32
ROUTED_SCALE = 2.5
NCORES = 8
NJ = T // 128              # 16 token tiles (r = p*16 + j)
TMAX = 9                   # static FFN tiles per core
NSL = 5                    # weight slots per core
SLOT_CAP = [4, 2, 1, 1, 1]
SLOT_TILES = [[0, 1, 2, 3], [4, 5], [6], [7], [8]]
NSLOT = TMAX * 128         # 1152 dispatch slots per core
LW = 2 + NE                # inversion lhsT width: r_hi, r_lo, 40 gates
NTA = 4                    # tau-group A: taus 0..3 (slot 0)
NTB = TMAX - NTA           # tau-group B: taus 4..8 (slots 1..4)
AluOp = mybir.AluOpType
ACT_F = mybir.ActivationFunctionType
AXL = mybir.AxisListType


# ---------------------------------------------------------------------------
# host-side schedule
# ---------------------------------------------------------------------------

def _host_routing(hidden, router_w, bias):
    """fp32 routing on host — used ONLY for load-balance scheduling."""
    logits = hidden.astype(np.float32) @ router_w.astype(np.float32).T
    m = logits.max(axis=1, keepdims=True)
    e = np.exp(logits - m)
    scores = e / e.sum(axis=1, keepdims=True)
    biased = scores + bias[None, :]
    ids = np.argsort(-biased, axis=1, kind="stable")[:, :4]
    return ids


def _schedule(ids):
    """Static tile schedule: split-anywhere first-fit-decreasing packing.

    Returns per-core:
      slot_expert[c][s]: global expert id serviced by local weight slot s
      tiles[c][tau]: (expert_id, lo_rank) — dispatch range for FFN tile tau
    Ranks are positions within an expert's selected-token list in r-order.
    """
    counts = np.zeros(ER, np.int64)
    for row in ids:
        for e in row:
            if e < ER:
                counts[e] += 1
    pieces = [[e, 0, (int(counts[e]) + 127) // 128] for e in range(ER)
              if counts[e] > 0]               # [expert, first_tile, ntiles]
    pieces.sort(key=lambda p: -p[2])
    slots = sorted(((SLOT_CAP[s], c, s) for c in range(NCORES)
                    for s in range(NSL)), key=lambda x: -x[0])
    slot_expert = [[0] * NSL for _ in range(NCORES)]
    tiles = [[(0, 1 << 14)] * TMAX for _ in range(NCORES)]
    si = 0
    work = []
    for p in pieces:
        work.append(p)
    while work:
        work.sort(key=lambda p: -p[2])
        p = work.pop(0)
        if si >= len(slots):
            raise RuntimeError("schedule: out of weight slots")
        cap, c, s = slots[si]
        si += 1
        take = min(cap, p[2])
        slot_expert[c][s] = p[0]
        for k in range(cap):
            tau = SLOT_TILES[s][k]
            # tiles beyond `take` extend the range as harmless slack
            tiles[c][tau] = (p[0], 128 * (p[1] + min(k, take)))
        for k in range(take):
            tiles[c][SLOT_TILES[s][k]] = (p[0], 128 * (p[1] + k))
        if p[2] > take:
            work.append([p[0], p[1] + take, p[2] - take])
    return slot_expert, tiles


# ---------------------------------------------------------------------------
# device graph
# ---------------------------------------------------------------------------

_NC_CACHE = {}


def build_nc():
    key = "v4"
    if key in _NC_CACHE:
        return _NC_CACHE[key]
    nc = bacc.Bacc("TRN2", target_bir_lowering=False, debug=False,
                   num_devices=NCORES)

    def din(name, shape, dt):
        return nc.dram_tensor(name, shape, dt, kind="ExternalInput").ap()

    hidden_Tt = din("hidden_Tt", [8, 128, 8, 256], F32)  # router chunks
    hidden_bf = din("hidden_bf", [T, H], BF16)         # r-ordered rows, bf16
    rwt = din("rwt", [H, NE], F32)                     # router_w.T
    bias_b = din("bias_b", [128, NE], F32)             # bias replicated
    w13s = din("w13s", [NSL, 128, 8, 2 * I], BF16)     # [slot, p, k, 2i]
    w2s = din("w2s", [NSL, 128, 4, H], BF16)           # [slot, p, k, h]
    tile_e = din("tile_e", [128, TMAX], F32)           # expert id per tile
    tile_lo = din("tile_lo", [128, TMAX], F32)         # rank range lo per tile
    rhl = din("rhl", [128, NJ, 2], BF16)               # r split (r//128, r%128)
    iota42m2 = din("iota42m2", [128, TMAX, LW], F32)   # value = col - 2
    iota128r = din("iota128r", [128, TMAX, 128], BF16) # value = col (0..127)
    ident = din("ident", [128, 128], F32)
    uts128 = din("uts128", [128, 128], F32)            # strict upper: [k,m]=k<m
    rep16 = din("rep16", [16, 128], F32)               # rep16[q,p] = (p%16==q)
    sel8 = din("sel8", [128, 8, 16], F32)              # sel8[p,g,q] = (p==16g+q)
    hz = din("hz", [256, H], F32)                      # my zero-path rows
    seqidx = din("seqidx", [128, 16], I16)             # my zero-path idxs

    partial = nc.dram_tensor("partial", [T, H], BF16,
                             kind="ExternalOutput").ap()

    with tile.TileContext(nc) as tc:
        with (
            tc.tile_pool(name="const", bufs=1) as cpool,
            tc.tile_pool(name="work", bufs=2) as wpool,
            tc.tile_pool(name="persist", bufs=1) as ppool,
            tc.tile_pool(name="wload", bufs=2) as wlpool,
            tc.tile_pool(name="hts", bufs=3) as htpool,
            tc.tile_pool(name="psum", bufs=2, space="PSUM") as pspool,
            tc.tile_pool(name="psumI", bufs=1, space="PSUM") as psipool,
            tc.tile_pool(name="psumA", bufs=2, space="PSUM") as psapool,
            tc.tile_pool(name="dram", bufs=1, space="DRAM") as dpool,
        ):
            nc.gpsimd.load_library(library_config.mlp)

            # ---- router weights + token chunks first on the sync queue ----
            rw_sb = cpool.tile([128, 8, NE], F32, tag="rw")
            nc.sync.dma_start(rw_sb[:],
                              rwt.rearrange("(k p) n -> p k n", p=128))
            ident_sb = cpool.tile([128, 128], F32, tag="ident")
            nc.sync.dma_start(ident_sb[:], ident[:])
            hts = []
            for cq in range(8):
                ht = htpool.tile([128, 8, 256], F32, tag="ht",
                                 name=f"ht{cq}")
                nc.sync.dma_start(ht[:], hidden_Tt[cq])
                hts.append(ht)

            # ---- resident constants (gpsimd DMA queue) ----
            bias_sb = cpool.tile([128, NE], F32, tag="bias")
            nc.gpsimd.dma_start(bias_sb[:], bias_b[:])
            uts_sb = cpool.tile([128, 128], F32, tag="uts")
            nc.gpsimd.dma_start(uts_sb[:], uts128[:])
            te_sb = cpool.tile([128, TMAX], F32, tag="te")
            nc.gpsimd.dma_start(te_sb[:], tile_e[:])
            tlo_sb = cpool.tile([128, TMAX], F32, tag="tlo")
            nc.gpsimd.dma_start(tlo_sb[:], tile_lo[:])
            i42_sb = cpool.tile([128, TMAX, LW], F32, tag="i42")
            nc.gpsimd.dma_start(i42_sb[:], iota42m2[:])
            i128_sb = cpool.tile([128, TMAX, 128], BF16, tag="i128")
            nc.gpsimd.dma_start(i128_sb[:], iota128r[:])
            rep16_sb = cpool.tile([16, 128], F32, tag="rep16")
            nc.gpsimd.dma_start(rep16_sb[:], rep16[:])
            sel8_sb = cpool.tile([128, 8, 16], F32, tag="sel8")
            nc.gpsimd.dma_start(sel8_sb[:], sel8[:])
            seq_sb = cpool.tile([128, 16], I16, tag="seqsb")
            nc.gpsimd.dma_start(seq_sb[:], seqidx[:])
            zeros16 = cpool.tile([128, NJ], F32, tag="z16")
            nc.vector.memset(zeros16[:], 0.0)

            # expert one-hot per tile: oh_te_f[p, tau, 2+e] = (e == te[tau])
            oh_te_f = cpool.tile([128, TMAX, LW], F32, tag="ohtef")
            nc.vector.tensor_tensor(
                oh_te_f[:], i42_sb[:],
                te_sb[:].unsqueeze(2).to_broadcast([128, TMAX, LW]),
                op=AluOp.is_equal)
            oh_te_b = cpool.tile([128, TMAX, LW], BF16, tag="ohteb")
            nc.vector.tensor_copy(oh_te_b[:], oh_te_f[:])

            # ---- persistent intermediates ----
            lhsT_all = ppool.tile([128, NJ, LW], BF16, tag="lhsT")
            nc.gpsimd.dma_start(lhsT_all[:, :, 0:2], rhl[:])
            sc3 = ppool.tile([128, NJ, NE], F32, tag="sc3")
            scores = ppool.tile([128, NJ, NE], F32, tag="scores")
            biased = ppool.tile([128, NJ, NE], F32, tag="biased")
            thr = ppool.tile([128, NJ, 1], F32, tag="thr")
            mask3 = ppool.tile([128, NJ, NE], F32, tag="mask3")
            mask3b = ppool.tile([128, NJ, NE], BF16, tag="mask3b")
            mg = ppool.tile([128, NJ, NE], F32, tag="mg")
            zt_all = ppool.tile([128, NJ], F32, tag="zt")
            mask_tau = ppool.tile([128, TMAX, NJ], F32, tag="masktau")

            # ---- PE warm-up (HAM) on the router weights ----
            for w in range(24):
                ps_w = pspool.tile([128, 128], F32, tag="ps_tr",
                                   name=f"ps_warm{w}")
                nc.tensor.matmul(ps_w[:NE, :], lhsT=rw_sb[:, 0, :],
                                 rhs=ident_sb[:], start=True, stop=True)

            # =============== phase 1: router (exact fp32) ===============
            wgate = dpool.tile([1, 1], F32, tag="wgate")
            for cq in range(8):
                ht = hts[cq]
                ps_lg = psapool.tile([40, 256], F32, tag="ps_big",
                                     name=f"ps_lg{cq}")
                for k in range(8):
                    nc.tensor.matmul(ps_lg[:], lhsT=rw_sb[:, k, :],
                                     rhs=ht[:, k, :],
                                     start=(k == 0), stop=(k == 7))
                lgs = wpool.tile([40, 256], F32, tag="lgs")
                nc.vector.tensor_copy(lgs[:], ps_lg[:])
                for q in range(2):
                    j = cq * 2 + q
                    ps_l = pspool.tile([128, 128], F32, tag="ps_tr",
                                       name=f"ps_lt{j}")
                    nc.tensor.transpose(ps_l[:, :NE],
                                        lgs[:, q * 128:(q + 1) * 128],
                                        ident_sb[:NE, :NE])
                    nc.vector.tensor_copy(sc3[:, j, :], ps_l[:, :NE])
                if cq == 0:
                    # gate the weight stream behind the router start
                    nc.scalar.dma_start(wgate[:], sc3[0:1, 0, 0:1])
                    wpre = []
                    for s in range(2):
                        w13_sb = wlpool.tile([128, 8, 2 * I], BF16, tag="w13",
                                             name=f"w13_s{s}")
                        w2_sb = wlpool.tile([128, 4, H], BF16, tag="w2",
                                            name=f"w2_s{s}")
                        nc.scalar.dma_start(w13_sb[:], w13s[s])
                        nc.scalar.dma_start(w2_sb[:], w2s[s])
                        wpre.append((w13_sb, w2_sb))
                if cq % 2 == 0:
                    continue
                g = cq // 2
                js = slice(g * 4, g * 4 + 4)
                sh = [128, 4, NE]
                rmax = wpool.tile([128, 4, 1], F32, tag="rmax")
                nc.vector.tensor_reduce(rmax[:], sc3[:, js], axis=AXL.X,
                                        op=AluOp.max, negate=True)  # -max
                xs = wpool.tile(sh, F32, tag="xs")
                nc.vector.tensor_tensor(xs[:], sc3[:, js],
                                        rmax[:].to_broadcast(sh),
                                        op=AluOp.add)
                exv = wpool.tile(sh, F32, tag="ex")
                nc.scalar.activation(exv[:], xs[:], ACT_F.Exp)
                rsum = wpool.tile([128, 4, 1], F32, tag="rsum")
                nc.vector.tensor_reduce(rsum[:], exv[:], axis=AXL.X,
                                        op=AluOp.add)
                rinv = wpool.tile([128, 4, 1], F32, tag="rinv")
                nc.vector.reciprocal(rinv[:], rsum[:])
                nc.vector.tensor_tensor(scores[:, js], exv[:],
                                        rinv[:].to_broadcast(sh),
                                        op=AluOp.mult)
                nc.vector.tensor_tensor(
                    biased[:, js], scores[:, js],
                    bias_sb[:].unsqueeze(1).to_broadcast(sh),
                    op=AluOp.add)
                for j in range(g * 4, g * 4 + 4):
                    t8 = wpool.tile([128, 8], F32, tag="t8")
                    nc.vector.max(t8[:], biased[:, j])
                    nc.vector.tensor_copy(thr[:, j, :], t8[:, 3:4])
                nc.vector.tensor_tensor(mask3[:, js], biased[:, js],
                                        thr[:, js].to_broadcast(sh),
                                        op=AluOp.is_ge)
                nc.vector.scalar_tensor_tensor(mg[:, js], mask3[:, js],
                                               ROUTED_SCALE, scores[:, js],
                                               op0=AluOp.mult, op1=AluOp.mult)
                nc.vector.tensor_reduce(zt_all[:, js], mg[:, js, ER:NE],
                                        axis=AXL.X, op=AluOp.add)
                nc.vector.tensor_copy(lhsT_all[:, js, 2:LW], mg[:, js])
                nc.vector.tensor_copy(mask3b[:, js], mask3[:, js])
                # tile membership for this 4-j group (dual broadcast)
                mtmp = wpool.tile([128, TMAX, 4, NE], BF16, tag="mtmp")
                nc.vector.tensor_tensor(
                    mtmp[:],
                    mask3b[:, js].unsqueeze(1).to_broadcast(
                        [128, TMAX, 4, NE]),
                    oh_te_b[:, :, 2:LW].unsqueeze(2).to_broadcast(
                        [128, TMAX, 4, NE]),
                    op=AluOp.mult)
                nc.vector.tensor_reduce(mask_tau[:, :, js], mtmp[:],
                                        axis=AXL.X, op=AluOp.add)

            # =============== zero-expert path (early) ===============
            zt_flat = dpool.tile([1, T], F32, tag="ztflat")
            nc.sync.dma_start(zt_flat[0, :].rearrange("(p j) -> p j", p=128),
                              zt_all[:])
            pid = nc.sync.partition_id()
            yz = ppool.tile([128, 2, H], BF16, tag="yz")
            for tt in range(2):
                ztv = wpool.tile([1, 128], F32, tag="ztv")
                nc.sync.dma_start(
                    ztv[:], zt_flat[0:1, bass.ds(pid * 256 + tt * 128, 128)])
                ps_zt = psapool.tile([128, 1], F32, tag="ps_big",
                                     name=f"ps_zt{tt}")
                nc.tensor.transpose(ps_zt[:], ztv[:], ident_sb[:1, :1])
                ztc = wpool.tile([128, 1], F32, tag="ztc")
                nc.vector.tensor_copy(ztc[:], ps_zt[:])
                hzt = wpool.tile([128, H], F32, tag="hzt")
                nc.sync.dma_start(hzt[:], hz[tt * 128:(tt + 1) * 128, :])
                nc.scalar.activation(yz[:, tt, :], hzt[:], ACT_F.Copy,
                                     scale=ztc[:, 0:1])
            nc.gpsimd.dma_scatter_add(
                out_ap=partial[:], in_ap=yz[:], idxs_ap=seq_sb[:],
                num_idxs=256, num_idxs_reg=256, elem_size=H)

            # =============== phase 2: ranks ===============
            inrow = wpool.tile([128, TMAX, NJ], F32, tag="inrow")
            for tau in range(TMAX):
                nc.vector.tensor_tensor_scan(inrow[:, tau], mask_tau[:, tau],
                                             zeros16[:], 0.0,
                                             op0=AluOp.add, op1=AluOp.add)
            rowsum = wpool.tile([128, TMAX], F32, tag="rowsum")
            nc.vector.tensor_reduce(rowsum[:], mask_tau[:], axis=AXL.X,
                                    op=AluOp.add)
            ps_rp = psapool.tile([128, TMAX], F32, tag="ps_big",
                                 name="ps_rp")
            nc.tensor.matmul(ps_rp[:], lhsT=uts_sb[:], rhs=rowsum[:],
                             start=True, stop=True)
            pref = wpool.tile([128, TMAX, 1], F32, tag="pref")
            nc.vector.tensor_copy(pref[:, :, 0], ps_rp[:])
            pos = wpool.tile([128, TMAX, NJ], F32, tag="pos")
            nc.vector.tensor_tensor(pos[:], inrow[:],
                                    pref[:].to_broadcast([128, TMAX, NJ]),
                                    op=AluOp.add)
            nc.vector.tensor_sub(pos[:], pos[:], mask_tau[:])
            t1 = wpool.tile([128, TMAX, NJ], F32, tag="t1")
            nc.vector.tensor_tensor(
                t1[:], pos[:],
                tlo_sb[:].unsqueeze(2).to_broadcast([128, TMAX, NJ]),
                op=AluOp.subtract)
            okr = wpool.tile([128, TMAX, NJ], F32, tag="okr")
            nc.vector.tensor_scalar(okr[:], t1[:], -0.5, None, op0=AluOp.is_gt)
            ok2 = wpool.tile([128, TMAX, NJ], F32, tag="ok2")
            nc.vector.tensor_scalar(ok2[:], t1[:], 127.5, None, op0=AluOp.is_lt)
            nc.vector.tensor_mul(okr[:], okr[:], ok2[:])
            nc.vector.tensor_mul(okr[:], okr[:], mask_tau[:])
            oku = wpool.tile([128, TMAX, NJ], U8, tag="oku")
            nc.vector.tensor_copy(oku[:], okr[:])
            t1m = wpool.tile([128, TMAX, NJ], F32, tag="t1m")
            nc.vector.memset(t1m[:], -4.0)
            nc.vector.copy_predicated(t1m[:], oku[:], t1[:])
            t1mb = ppool.tile([128, TMAX, NJ], BF16, tag="t1mb")
            nc.vector.tensor_copy(t1mb[:], t1m[:])

            # =============== phase 3: split inversion + decode ===============
            tsp = ppool.tile([128, TMAX, LW], F32, tag="tsp")
            r_pt = ppool.tile([128, TMAX], F32, tag="rpt")
            g_wr = ppool.tile([128, TMAX], F32, tag="gwr")
            idxw = ppool.tile([128, 8 * TMAX], I16, tag="idxw")
            xts = {}

            for half, (tl, th_) in enumerate([(0, NTA), (NTA, TMAX)]):
                ntau = th_ - tl
                inv_ps = psipool.tile([LW, ntau * 128], F32, tag="ps_inv",
                                      name=f"ps_inv{half}")
                for j in range(NJ):
                    oh = wpool.tile([128, ntau, 128], BF16, tag=f"oh{half}")
                    nc.vector.tensor_tensor(
                        oh[:],
                        t1mb[:, tl:th_, j].unsqueeze(2).to_broadcast(
                            [128, ntau, 128]),
                        i128_sb[:, tl:th_, :],
                        op=AluOp.is_equal)
                    ohf = oh[:].rearrange("p a b -> p (a b)")
                    for lo in range(0, ntau * 128, 512):
                        hi = min(lo + 512, ntau * 128)
                        nc.tensor.matmul(inv_ps[:, lo:hi],
                                         lhsT=lhsT_all[:, j],
                                         rhs=ohf[:, lo:hi],
                                         start=(j == 0), stop=(j == NJ - 1))
                inv_sb = ppool.tile([LW, ntau * 128], F32, tag=f"invsb{half}")
                nc.vector.tensor_copy(inv_sb[:], inv_ps[:])
                for ti in range(ntau):
                    tau = tl + ti
                    ps_tsp = pspool.tile([128, 128], F32, tag="ps_tr",
                                         name=f"ps_tsp{tau}")
                    nc.tensor.transpose(ps_tsp[:, :LW],
                                        inv_sb[:, ti * 128:(ti + 1) * 128],
                                        ident_sb[:LW, :LW])
                    nc.vector.tensor_copy(tsp[:, tau], ps_tsp[:, :LW])
                nc.vector.scalar_tensor_tensor(
                    r_pt[:, tl:th_], tsp[:, tl:th_, 0], 128.0,
                    tsp[:, tl:th_, 1], op0=AluOp.mult, op1=AluOp.add)
                gtmp = wpool.tile([128, ntau, LW], F32, tag=f"gtmp{half}")
                nc.vector.tensor_tensor(gtmp[:], tsp[:, tl:th_],
                                        oh_te_f[:, tl:th_], op=AluOp.mult)
                nc.vector.tensor_reduce(g_wr[:, tl:th_], gtmp[:], axis=AXL.X,
                                        op=AluOp.add)
                idx16 = wpool.tile([16, 8 * ntau], F32, tag=f"idx16{half}")
                for g in range(8):
                    ps_g = psapool.tile([16, ntau], F32, tag="ps_big",
                                        name=f"ps_selg{half}_{g}")
                    nc.tensor.matmul(ps_g[:], lhsT=sel8_sb[:, g],
                                     rhs=r_pt[:, tl:th_],
                                     start=True, stop=True)
                    nc.vector.tensor_copy(idx16[:, g::8], ps_g[:])
                ps_rep = psapool.tile([128, 8 * ntau], F32, tag="ps_big",
                                      name=f"ps_rep{half}")
                nc.tensor.matmul(ps_rep[:], lhsT=rep16_sb[:], rhs=idx16[:],
                                 start=True, stop=True)
                nc.vector.tensor_copy(idxw[:, tl * 8:th_ * 8], ps_rep[:])

                # gathers for this half's slots
                for s in ([0] if half == 0 else [1, 2, 3, 4]):
                    nt = SLOT_CAP[s]
                    t0 = SLOT_TILES[s][0]
                    xt = wlpool.tile([128, 8, nt * 128], BF16, tag=f"xt{nt}",
                                     name=f"xt_s{s}")
                    nc.gpsimd.dma_gather(
                        out_ap=xt[:], in_ap=hidden_bf[:],
                        idxs_ap=idxw[:, t0 * 8:(t0 + nt) * 8],
                        num_idxs=nt * 128, num_idxs_reg=nt * 128,
                        elem_size=H, transpose=True)
                    xts[s] = xt

                # FFN for this half's slots
                for s in ([0] if half == 0 else [1, 2, 3, 4]):
                    nt = SLOT_CAP[s]
                    xt = xts[s]
                    if s < 2:
                        w13_sb, w2_sb = wpre[s]
                    else:
                        w13_sb = wlpool.tile([128, 8, 2 * I], BF16,
                                             tag="w13", name=f"w13l_s{s}")
                        w2_sb = wlpool.tile([128, 4, H], BF16, tag="w2",
                                            name=f"w2l_s{s}")
                        nc.scalar.dma_start(w13_sb[:], w13s[s])
                        nc.scalar.dma_start(w2_sb[:], w2s[s])
                    groups = [SLOT_TILES[s][i:i + 2]
                              for i in range(0, nt, 2)]
                    for gi, grp in enumerate(groups):
                        gn = len(grp)
                        gt0 = grp[0]
                        yv = wpool.tile([128, gn, H], BF16, tag=f"yv{gn}",
                                        name=f"yv_s{s}g{gi}")
                        for ti, tau in enumerate(grp):
                            xti = SLOT_TILES[s].index(tau)
                            ps_gu = psapool.tile([128, 2 * I], F32,
                                                 tag="ps_big",
                                                 name=f"ps_gu{tau}")
                            for k in range(8):
                                for n in range(2):
                                    nc.tensor.matmul(
                                        ps_gu[:, n * 512:(n + 1) * 512],
                                        lhsT=xt[:, k,
                                                xti * 128:(xti + 1) * 128],
                                        rhs=w13_sb[:, k,
                                                   n * 512:(n + 1) * 512],
                                        start=(k == 0), stop=(k == 7))
                            sl = wpool.tile([128, I], F32, tag="sl")
                            nc.scalar.activation(sl[:], ps_gu[:, :I],
                                                 ACT_F.Silu)
                            hh = wpool.tile([128, I], F32, tag="hh")
                            nc.vector.tensor_mul(hh[:], sl[:], ps_gu[:, I:])
                            hT = wpool.tile([128, 4, 128], BF16, tag="hT")
                            for k in range(4):
                                ps_t2 = pspool.tile([128, 128], F32,
                                                    tag="ps_tr")
                                nc.tensor.transpose(
                                    ps_t2[:], hh[:, k * 128:(k + 1) * 128],
                                    ident_sb[:])
                                if k % 2 == 0:
                                    nc.vector.tensor_copy(hT[:, k], ps_t2[:])
                                else:
                                    nc.scalar.activation(hT[:, k], ps_t2[:],
                                                         ACT_F.Copy)
                            ps_y = psapool.tile([128, H], F32, tag="ps_big",
                                                name=f"ps_y{tau}")
                            for k in range(4):
                                for n in range(2):
                                    nc.tensor.matmul(
                                        ps_y[:, n * 512:(n + 1) * 512],
                                        lhsT=hT[:, k],
                                        rhs=w2_sb[:, k,
                                                  n * 512:(n + 1) * 512],
                                        start=(k == 0), stop=(k == 3))
                            nc.vector.tensor_scalar(yv[:, ti, :], ps_y[:],
                                                    g_wr[:, tau:tau + 1],
                                                    None, op0=AluOp.mult)
                        nc.gpsimd.dma_scatter_add(
                            out_ap=partial[:], in_ap=yv[:],
                            idxs_ap=idxw[:, gt0 * 8:(gt0 + gn) * 8],
                            num_idxs=gn * 128, num_idxs_reg=gn * 128,
                            elem_size=H)

    nc.compile()
    _NC_CACHE[key] = nc
    return nc


# ---------------------------------------------------------------------------
# host wrapper
# ---------------------------------------------------------------------------

def make_in_maps(hidden_states, router_w, e_score_correction_bias, w13, w2):
    import ml_dtypes
    hidden_states = np.asarray(hidden_states, np.float32)
    router_w = np.asarray(router_w, np.float32)
    bias = np.asarray(e_score_correction_bias, np.float32)
    w13 = np.asarray(w13, np.float32)
    w2 = np.asarray(w2, np.float32)

    ids = _host_routing(hidden_states, router_w, bias)
    slot_expert, tiles = _schedule(ids)

    # r = (t % 128) * 16 + t // 128  <->  t = (r % 16) * 128 + r // 16
    r_of_t = (np.arange(T) % 128) * 16 + np.arange(T) // 128
    t_of_r = np.empty(T, np.int64)
    t_of_r[r_of_t] = np.arange(T)

    hidden_T = np.ascontiguousarray(hidden_states.T)
    # [k, p, cq, u] -> [cq, p, k, u]
    hidden_Tt = np.ascontiguousarray(
        hidden_T.reshape(8, 128, 8, 256).transpose(2, 1, 0, 3))
    hidden_rows = np.ascontiguousarray(hidden_states[t_of_r])
    hidden_bf = hidden_rows.astype(ml_dtypes.bfloat16)
    rwt = np.ascontiguousarray(router_w.T)
    bias_b = np.tile(bias[None, :], (128, 1))
    w13t = w13.transpose(0, 2, 1)                  # [e, h, 2I]
    w2t = w2.transpose(0, 2, 1)                    # [e, i, h]
    # host-tiled contiguous weight layout: [e, p, k, i]
    w13tt = np.ascontiguousarray(
        w13t.reshape(ER, 8, 128, 2 * I).transpose(0, 2, 1, 3)).astype(
            ml_dtypes.bfloat16)
    w2tt = np.ascontiguousarray(
        w2t.reshape(ER, 4, 128, H).transpose(0, 2, 1, 3)).astype(
            ml_dtypes.bfloat16)

    rr = np.arange(T).reshape(128, NJ).astype(np.float32)  # r at [p, j]
    rhl = np.stack([rr // 128, rr % 128], axis=-1).astype(ml_dtypes.bfloat16)
    iota42m2 = np.tile(np.arange(-2, NE, dtype=np.float32), (128, TMAX, 1))
    iota128r = np.tile(np.arange(128, dtype=np.float32), (128, TMAX, 1)) \
        .astype(ml_dtypes.bfloat16)
    ident = np.eye(128, dtype=np.float32)
    uts128 = np.triu(np.ones((128, 128), np.float32), k=1)
    rep16 = np.zeros((16, 128), np.float32)
    rep16[np.arange(128) % 16, np.arange(128)] = 1.0
    sel8 = np.zeros((128, 8, 16), np.float32)
    for g in range(8):
        sel8[16 * g + np.arange(16), g, np.arange(16)] = 1.0

    p_ = np.arange(128)[:, None]
    f_ = np.arange(16)[None, :]
    seq_base = (f_ % 8) * 16 + (p_ % 16) + (f_ // 8) * 128  # [p, f]

    in_maps = []
    for c in range(NCORES):
        te = np.array([tiles[c][tau][0] for tau in range(TMAX)], np.float32)
        tlo = np.array([tiles[c][tau][1] for tau in range(TMAX)], np.float32)
        in_maps.append({
            "hidden_Tt": hidden_Tt,
            "hidden_bf": hidden_bf,
            "rwt": rwt,
            "bias_b": bias_b,
            "w13s": np.ascontiguousarray(
                w13tt[[slot_expert[c][s] for s in range(NSL)]]),
            "w2s": np.ascontiguousarray(
                w2tt[[slot_expert[c][s] for s in range(NSL)]]),
            "tile_e": np.tile(te[None, :], (128, 1)),
            "tile_lo": np.tile(tlo[None, :], (128, 1)),
            "rhl": rhl,
            "iota42m2": iota42m2,
            "iota128r": iota128r,
            "ident": ident,
            "uts128": uts128,
            "rep16": rep16,
            "sel8": sel8,
            "hz": np.ascontiguousarray(hidden_rows[c * 256:(c + 1) * 256]),
            "seqidx": (seq_base + c * 256).astype(np.int16),
        })
    return in_maps, t_of_r


def kernel(hidden_states, router_w, e_score_correction_bias, w13, w2,
           _trace=False):
    nc = build_nc()
    in_maps, t_of_r = make_in_maps(hidden_states, router_w,
                                   e_score_correction_bias, w13, w2)
    res = run_bass_kernel_spmd(nc, in_maps, core_ids=list(range(NCORES)),
                               trace=_trace)
    total = np.zeros((T, H), np.float64)
    for c in range(NCORES):
        total += res.results[c]["partial"].astype(np.float64)
    out = np.empty((T, H), np.float32)
    out[t_of_r] = total.astype(np.float32)      # out[t] = total[r(t)]
    kernel._last_results = res
    return out


# revision 11
# speedup vs baseline: 1.5441x; 1.0279x over previous
"""LongcatFlash MoE kernel for 8 TRN2 NeuronCores (expert-parallel).

Contract: kernel(**inputs) takes the FULL un-sharded inputs from
reference.setup_inputs() and returns the FULL [T, H] output.

Strategy (expert-parallel, memory-regime), v4:
  - Router replicated, exact fp32 matmuls; PE pre-warmed with dummy
    matmuls so the stream runs at full clock. Softmax / top-4-threshold
    / mask / per-tile-membership vector work is chunked per 512 tokens
    and runs under the router matmul stream.
  - Selection is a mask (biased >= 4th-largest); gates flow as the
    masked-scaled score vector straight into the inversion matmul.
  - Dispatch inversion is split into two tau-groups: slot-0's tiles
    decode + gather + compute while the second group still inverts.
  - Slot->token lists built on-chip with permutation matmuls.
  - FFN: bf16 weights (DMA gated behind router start so the hidden
    chunks win early bandwidth), transposed bf16 gather for x, Silu on
    the scalar engine, bf16 partial output with CCE scatter-add.
  - Zero-experts (ids >= 32) reduce to a per-token scale of the hidden
    row, applied by the token-range owner core.
  - Host unshards by summing the 8 partial outputs and undoing the row
    permutation r = (t % 128) * 16 + t // 128.
"""

import numpy as np

import concourse.bacc as bacc
import concourse.bass as bass
import concourse.mybir as mybir
import concourse.tile as tile
from concourse import library_config
from concourse.bass_utils import run_bass_kernel_spmd

F32 = mybir.dt.float32
BF16 = mybir.dt.bfloat16
I16 = mybir.dt.int16
U8 = mybir.dt.uint8

T, H, I = 2048, 1024, 512
NE, ER = 40, 32
ROUTED_SCALE = 2.5
NCORES = 8
NJ = T // 128              # 16 token tiles (r = p*16 + j)
TMAX = 9                   # static FFN tiles per core
NSL = 5                    # weight slots per core
SLOT_CAP = [4, 2, 1, 1, 1]
SLOT_TILES = [[0, 1, 2, 3], [4, 5], [6], [7], [8]]
NSLOT = TMAX * 128         # 1152 dispatch slots per core
LW = 2 + NE                # inversion lhsT width: r_hi, r_lo, 40 gates
NTA = 4                    # tau-group A: taus 0..3 (slot 0)
NTB = TMAX - NTA           # tau-group B: taus 4..8 (slots 1..4)
AluOp = mybir.AluOpType
ACT_F = mybir.ActivationFunctionType
AXL = mybir.AxisListType


# ---------------------------------------------------------------------------
# host-side schedule
# ---------------------------------------------------------------------------

def _host_routing(hidden, router_w, bias):
    """fp32 routing on host — used ONLY for load-balance scheduling."""
    logits = hidden.astype(np.float32) @ router_w.astype(np.float32).T
    m = logits.max(axis=1, keepdims=True)
    e = np.exp(logits - m)
    scores = e / e.sum(axis=1, keepdims=True)
    biased = scores + bias[None, :]
    ids = np.argsort(-biased, axis=1, kind="stable")[:, :4]
    return ids


def _schedule(ids):
    """Static tile schedule: split-anywhere first-fit-decreasing packing.

    Returns per-core:
      slot_expert[c][s]: global expert id serviced by local weight slot s
      tiles[c][tau]: (expert_id, lo_rank) — dispatch range for FFN tile tau
    Ranks are positions within an expert's selected-token list in r-order.
    """
    counts = np.zeros(ER, np.int64)
    for row in ids:
        for e in row:
            if e < ER:
                counts[e] += 1
    pieces = [[e, 0, (int(counts[e]) + 127) // 128] for e in range(ER)
              if counts[e] > 0]               # [expert, first_tile, ntiles]
    pieces.sort(key=lambda p: -p[2])
    slots = sorted(((SLOT_CAP[s], c, s) for c in range(NCORES)
                    for s in range(NSL)), key=lambda x: -x[0])
    slot_expert = [[0] * NSL for _ in range(NCORES)]
    tiles = [[(0, 1 << 14)] * TMAX for _ in range(NCORES)]
    si = 0
    work = []
    for p in pieces:
        work.append(p)
    while work:
        work.sort(key=lambda p: -p[2])
        p = work.pop(0)
        if si >= len(slots):
            raise RuntimeError("schedule: out of weight slots")
        cap, c, s = slots[si]
        si += 1
        take = min(cap, p[2])
        slot_expert[c][s] = p[0]
        for k in range(cap):
            tau = SLOT_TILES[s][k]
            # tiles beyond `take` extend the range as harmless slack
            tiles[c][tau] = (p[0], 128 * (p[1] + min(k, take)))
        for k in range(take):
            tiles[c][SLOT_TILES[s][k]] = (p[0], 128 * (p[1] + k))
        if p[2] > take:
            work.append([p[0], p[1] + take, p[2] - take])
    return slot_expert, tiles


# ---------------------------------------------------------------------------
# device graph
# ---------------------------------------------------------------------------

_NC_CACHE = {}


def build_nc():
    key = "v4"
    if key in _NC_CACHE:
        return _NC_CACHE[key]
    nc = bacc.Bacc("TRN2", target_bir_lowering=False, debug=False,
                   num_devices=NCORES)

    def din(name, shape, dt):
        return nc.dram_tensor(name, shape, dt, kind="ExternalInput").ap()

    hidden_Tt = din("hidden_Tt", [8, 128, 8, 256], F32)  # router chunks
    hidden_bf = din("hidden_bf", [T, H], BF16)         # r-ordered rows, bf16
    rwt = din("rwt", [H, NE], F32)                     # router_w.T
    bias_b = din("bias_b", [128, NE], F32)             # bias replicated
    w13s = din("w13s", [NSL, 128, 8, 2 * I], BF16)     # [slot, p, k, 2i]
    w2s = din("w2s", [NSL, 128, 4, H], BF16)           # [slot, p, k, h]
    tile_e = din("tile_e", [128, TMAX], F32)           # expert id per tile
    tile_lo = din("tile_lo", [128, TMAX], F32)         # rank range lo per tile
    rhl = din("rhl", [128, NJ, 2], BF16)               # r split (r//128, r%128)
    iota42m2 = din("iota42m2", [128, TMAX, LW], F32)   # value = col - 2
    iota128r = din("iota128r", [128, TMAX, 128], BF16) # value = col (0..127)
    ident = din("ident", [128, 128], F32)
    uts128 = din("uts128", [128, 128], F32)            # strict upper: [k,m]=k<m
    rep16 = din("rep16", [16, 128], F32)               # rep16[q,p] = (p%16==q)
    sel8 = din("sel8", [128, 8, 16], F32)              # sel8[p,g,q] = (p==16g+q)
    hz = din("hz", [256, H], F32)                      # my zero-path rows
    seqidx = din("seqidx", [128, 16], I16)             # my zero-path idxs

    partial = nc.dram_tensor("partial", [T, H], BF16,
                             kind="ExternalOutput").ap()

    with tile.TileContext(nc) as tc:
        with (
            tc.tile_pool(name="const", bufs=1) as cpool,
            tc.tile_pool(name="work", bufs=2) as wpool,
            tc.tile_pool(name="persist", bufs=1) as ppool,
            tc.tile_pool(name="wload", bufs=2) as wlpool,
            tc.tile_pool(name="hts", bufs=3) as htpool,
            tc.tile_pool(name="gt", bufs=1) as gtpool,
            tc.tile_pool(name="psum", bufs=2, space="PSUM") as pspool,
            tc.tile_pool(name="psumI", bufs=1, space="PSUM") as psipool,
            tc.tile_pool(name="psumA", bufs=2, space="PSUM") as psapool,
            tc.tile_pool(name="dram", bufs=1, space="DRAM") as dpool,
        ):
            nc.gpsimd.load_library(library_config.mlp)

            # ---- router weights + token chunks first on the sync queue ----
            rw_sb = cpool.tile([128, 8, NE], F32, tag="rw")
            nc.sync.dma_start(rw_sb[:],
                              rwt.rearrange("(k p) n -> p k n", p=128))
            ident_sb = cpool.tile([128, 128], F32, tag="ident")
            nc.sync.dma_start(ident_sb[:], ident[:])
            hts = []
            for cq in range(8):
                ht = htpool.tile([128, 8, 256], F32, tag="ht",
                                 name=f"ht{cq}")
                nc.sync.dma_start(ht[:], hidden_Tt[cq])
                hts.append(ht)

            # ---- resident constants (gpsimd DMA queue) ----
            bias_sb = cpool.tile([128, NE], F32, tag="bias")
            nc.gpsimd.dma_start(bias_sb[:], bias_b[:])
            uts_sb = cpool.tile([128, 128], F32, tag="uts")
            nc.gpsimd.dma_start(uts_sb[:], uts128[:])
            te_sb = cpool.tile([128, TMAX], F32, tag="te")
            nc.gpsimd.dma_start(te_sb[:], tile_e[:])
            tlo_sb = cpool.tile([128, TMAX], F32, tag="tlo")
            nc.gpsimd.dma_start(tlo_sb[:], tile_lo[:])
            i42_sb = cpool.tile([128, TMAX, LW], F32, tag="i42")
            nc.gpsimd.dma_start(i42_sb[:], iota42m2[:])
            i128_sb = cpool.tile([128, TMAX, 128], BF16, tag="i128")
            nc.gpsimd.dma_start(i128_sb[:], iota128r[:])
            rep16_sb = cpool.tile([16, 128], F32, tag="rep16")
            nc.gpsimd.dma_start(rep16_sb[:], rep16[:])
            sel8_sb = cpool.tile([128, 8, 16], F32, tag="sel8")
            nc.gpsimd.dma_start(sel8_sb[:], sel8[:])
            seq_sb = cpool.tile([128, 16], I16, tag="seqsb")
            nc.gpsimd.dma_start(seq_sb[:], seqidx[:])
            zeros16 = cpool.tile([128, NJ], F32, tag="z16")
            nc.vector.memset(zeros16[:], 0.0)

            # expert one-hot per tile: oh_te_f[p, tau, 2+e] = (e == te[tau])
            oh_te_f = cpool.tile([128, TMAX, LW], F32, tag="ohtef")
            nc.vector.tensor_tensor(
                oh_te_f[:], i42_sb[:],
                te_sb[:].unsqueeze(2).to_broadcast([128, TMAX, LW]),
                op=AluOp.is_equal)
            oh_te_b = cpool.tile([128, TMAX, LW], BF16, tag="ohteb")
            nc.vector.tensor_copy(oh_te_b[:], oh_te_f[:])

            # ---- persistent intermediates ----
            lhsT_all = ppool.tile([128, NJ, LW], BF16, tag="lhsT")
            nc.gpsimd.dma_start(lhsT_all[:, :, 0:2], rhl[:])
            sc3 = ppool.tile([128, NJ, NE], F32, tag="sc3")
            scores = ppool.tile([128, NJ, NE], F32, tag="scores")
            biased = ppool.tile([128, NJ, NE], F32, tag="biased")
            thr = ppool.tile([128, NJ, 1], F32, tag="thr")
            mask3 = ppool.tile([128, NJ, NE], F32, tag="mask3")
            mask3b = ppool.tile([128, NJ, NE], BF16, tag="mask3b")
            mg = ppool.tile([128, NJ, NE], F32, tag="mg")
            zt_all = ppool.tile([128, NJ], F32, tag="zt")
            mask_tau = ppool.tile([128, TMAX, NJ], F32, tag="masktau")

            # ---- PE warm-up (HAM) on the router weights ----
            for w in range(24):
                ps_w = pspool.tile([128, 128], F32, tag="ps_tr",
                                   name=f"ps_warm{w}")
                nc.tensor.matmul(ps_w[:NE, :], lhsT=rw_sb[:, 0, :],
                                 rhs=ident_sb[:], start=True, stop=True)

            # =============== phase 1: router (exact fp32) ===============
            wgate = dpool.tile([1, 1], F32, tag="wgate")
            for cq in range(8):
                ht = hts[cq]
                ps_lg = psapool.tile([40, 256], F32, tag="ps_big",
                                     name=f"ps_lg{cq}")
                for k in range(8):
                    nc.tensor.matmul(ps_lg[:], lhsT=rw_sb[:, k, :],
                                     rhs=ht[:, k, :],
                                     start=(k == 0), stop=(k == 7))
                lgs = wpool.tile([40, 256], F32, tag="lgs")
                nc.vector.tensor_copy(lgs[:], ps_lg[:])
                for q in range(2):
                    j = cq * 2 + q
                    ps_l = pspool.tile([128, 128], F32, tag="ps_tr",
                                       name=f"ps_lt{j}")
                    nc.tensor.transpose(ps_l[:, :NE],
                                        lgs[:, q * 128:(q + 1) * 128],
                                        ident_sb[:NE, :NE])
                    nc.vector.tensor_copy(sc3[:, j, :], ps_l[:, :NE])
                if cq == 6:
                    # gate the weight stream until most hidden chunks landed
                    nc.scalar.dma_start(wgate[:], sc3[0:1, 12, 0:1])
                    wpre = []
                    for s in range(2):
                        w13_sb = wlpool.tile([128, 8, 2 * I], BF16, tag="w13",
                                             name=f"w13_s{s}")
                        w2_sb = wlpool.tile([128, 4, H], BF16, tag="w2",
                                            name=f"w2_s{s}")
                        nc.scalar.dma_start(w13_sb[:], w13s[s])
                        nc.scalar.dma_start(w2_sb[:], w2s[s])
                        wpre.append((w13_sb, w2_sb))
                if cq % 2 == 0:
                    continue
                g = cq // 2
                js = slice(g * 4, g * 4 + 4)
                sh = [128, 4, NE]
                rmax = wpool.tile([128, 4, 1], F32, tag="rmax")
                nc.vector.tensor_reduce(rmax[:], sc3[:, js], axis=AXL.X,
                                        op=AluOp.max, negate=True)  # -max
                xs = wpool.tile(sh, F32, tag="xs")
                nc.vector.tensor_tensor(xs[:], sc3[:, js],
                                        rmax[:].to_broadcast(sh),
                                        op=AluOp.add)
                exv = wpool.tile(sh, F32, tag="ex")
                nc.scalar.activation(exv[:], xs[:], ACT_F.Exp)
                rsum = wpool.tile([128, 4, 1], F32, tag="rsum")
                nc.vector.tensor_reduce(rsum[:], exv[:], axis=AXL.X,
                                        op=AluOp.add)
                rinv = wpool.tile([128, 4, 1], F32, tag="rinv")
                nc.vector.reciprocal(rinv[:], rsum[:])
                nc.vector.tensor_tensor(scores[:, js], exv[:],
                                        rinv[:].to_broadcast(sh),
                                        op=AluOp.mult)
                nc.vector.tensor_tensor(
                    biased[:, js], scores[:, js],
                    bias_sb[:].unsqueeze(1).to_broadcast(sh),
                    op=AluOp.add)
                for j in range(g * 4, g * 4 + 4):
                    t8 = wpool.tile([128, 8], F32, tag="t8")
                    nc.vector.max(t8[:], biased[:, j])
                    nc.vector.tensor_copy(thr[:, j, :], t8[:, 3:4])
                nc.vector.tensor_tensor(mask3[:, js], biased[:, js],
                                        thr[:, js].to_broadcast(sh),
                                        op=AluOp.is_ge)
                nc.vector.scalar_tensor_tensor(mg[:, js], mask3[:, js],
                                               ROUTED_SCALE, scores[:, js],
                                               op0=AluOp.mult, op1=AluOp.mult)
                nc.vector.tensor_reduce(zt_all[:, js], mg[:, js, ER:NE],
                                        axis=AXL.X, op=AluOp.add)
                nc.vector.tensor_copy(lhsT_all[:, js, 2:LW], mg[:, js])
                nc.vector.tensor_copy(mask3b[:, js], mask3[:, js])
                # tile membership for this 4-j group (dual broadcast)
                mtmp = wpool.tile([128, TMAX, 4, NE], BF16, tag="mtmp")
                nc.vector.tensor_tensor(
                    mtmp[:],
                    mask3b[:, js].unsqueeze(1).to_broadcast(
                        [128, TMAX, 4, NE]),
                    oh_te_b[:, :, 2:LW].unsqueeze(2).to_broadcast(
                        [128, TMAX, 4, NE]),
                    op=AluOp.mult)
                nc.vector.tensor_reduce(mask_tau[:, :, js], mtmp[:],
                                        axis=AXL.X, op=AluOp.add)

            # =============== zero-expert path (scatter emitted later) ====
            hzts = []
            for tt in range(2):
                hzt = wpool.tile([128, H], F32, tag="hzt")
                nc.sync.dma_start(hzt[:], hz[tt * 128:(tt + 1) * 128, :])
                hzts.append(hzt)
            zt_flat = dpool.tile([1, T], F32, tag="ztflat")
            nc.sync.dma_start(zt_flat[0, :].rearrange("(p j) -> p j", p=128),
                              zt_all[:])
            pid = nc.sync.partition_id()
            yz = ppool.tile([128, 2, H], BF16, tag="yz")
            for tt in range(2):
                ztv = wpool.tile([1, 128], F32, tag="ztv")
                nc.sync.dma_start(
                    ztv[:], zt_flat[0:1, bass.ds(pid * 256 + tt * 128, 128)])
                ps_zt = psapool.tile([128, 1], F32, tag="ps_big",
                                     name=f"ps_zt{tt}")
                nc.tensor.transpose(ps_zt[:], ztv[:], ident_sb[:1, :1])
                ztc = wpool.tile([128, 1], F32, tag="ztc")
                nc.vector.tensor_copy(ztc[:], ps_zt[:])
                nc.scalar.activation(yz[:, tt, :], hzts[tt][:], ACT_F.Copy,
                                     scale=ztc[:, 0:1])

            # =============== phase 2: ranks ===============
            inrow = wpool.tile([128, TMAX, NJ], F32, tag="inrow")
            for tau in range(TMAX):
                nc.vector.tensor_tensor_scan(inrow[:, tau], mask_tau[:, tau],
                                             zeros16[:], 0.0,
                                             op0=AluOp.add, op1=AluOp.add)
            rowsum = wpool.tile([128, TMAX], F32, tag="rowsum")
            nc.vector.tensor_reduce(rowsum[:], mask_tau[:], axis=AXL.X,
                                    op=AluOp.add)
            ps_rp = psapool.tile([128, TMAX], F32, tag="ps_big",
                                 name="ps_rp")
            nc.tensor.matmul(ps_rp[:], lhsT=uts_sb[:], rhs=rowsum[:],
                             start=True, stop=True)
            pref = wpool.tile([128, TMAX, 1], F32, tag="pref")
            nc.vector.tensor_copy(pref[:, :, 0], ps_rp[:])
            pos = wpool.tile([128, TMAX, NJ], F32, tag="pos")
            nc.vector.tensor_tensor(pos[:], inrow[:],
                                    pref[:].to_broadcast([128, TMAX, NJ]),
                                    op=AluOp.add)
            nc.vector.tensor_sub(pos[:], pos[:], mask_tau[:])
            t1 = wpool.tile([128, TMAX, NJ], F32, tag="t1")
            nc.vector.tensor_tensor(
                t1[:], pos[:],
                tlo_sb[:].unsqueeze(2).to_broadcast([128, TMAX, NJ]),
                op=AluOp.subtract)
            okr = wpool.tile([128, TMAX, NJ], F32, tag="okr")
            nc.vector.tensor_scalar(okr[:], t1[:], -0.5, None, op0=AluOp.is_gt)
            ok2 = wpool.tile([128, TMAX, NJ], F32, tag="ok2")
            nc.vector.tensor_scalar(ok2[:], t1[:], 127.5, None, op0=AluOp.is_lt)
            nc.vector.tensor_mul(okr[:], okr[:], ok2[:])
            nc.vector.tensor_mul(okr[:], okr[:], mask_tau[:])
            oku = wpool.tile([128, TMAX, NJ], U8, tag="oku")
            nc.vector.tensor_copy(oku[:], okr[:])
            t1m = wpool.tile([128, TMAX, NJ], F32, tag="t1m")
            nc.vector.memset(t1m[:], -4.0)
            nc.vector.copy_predicated(t1m[:], oku[:], t1[:])
            t1mb = ppool.tile([128, TMAX, NJ], BF16, tag="t1mb")
            nc.vector.tensor_copy(t1mb[:], t1m[:])

            # =============== phase 3: split inversion + decode ===============
            tsp = ppool.tile([128, TMAX, LW], F32, tag="tsp")
            r_pt = ppool.tile([128, TMAX], F32, tag="rpt")
            g_wr = ppool.tile([128, TMAX], F32, tag="gwr")
            idxw = ppool.tile([128, 8 * TMAX], I16, tag="idxw")
            xts = {}

            for half, (tl, th_) in enumerate([(0, NTA), (NTA, TMAX)]):
                ntau = th_ - tl
                inv_ps = psipool.tile([LW, ntau * 128], F32, tag="ps_inv",
                                      name=f"ps_inv{half}")
                for j in range(NJ):
                    oh = wpool.tile([128, ntau, 128], BF16, tag=f"oh{half}")
                    nc.vector.tensor_tensor(
                        oh[:],
                        t1mb[:, tl:th_, j].unsqueeze(2).to_broadcast(
                            [128, ntau, 128]),
                        i128_sb[:, tl:th_, :],
                        op=AluOp.is_equal)
                    ohf = oh[:].rearrange("p a b -> p (a b)")
                    for lo in range(0, ntau * 128, 512):
                        hi = min(lo + 512, ntau * 128)
                        nc.tensor.matmul(inv_ps[:, lo:hi],
                                         lhsT=lhsT_all[:, j],
                                         rhs=ohf[:, lo:hi],
                                         start=(j == 0), stop=(j == NJ - 1))
                inv_sb = ppool.tile([LW, ntau * 128], F32, tag=f"invsb{half}")
                nc.vector.tensor_copy(inv_sb[:], inv_ps[:])
                for ti in range(ntau):
                    tau = tl + ti
                    ps_tsp = pspool.tile([128, 128], F32, tag="ps_tr",
                                         name=f"ps_tsp{tau}")
                    nc.tensor.transpose(ps_tsp[:, :LW],
                                        inv_sb[:, ti * 128:(ti + 1) * 128],
                                        ident_sb[:LW, :LW])
                    nc.vector.tensor_copy(tsp[:, tau], ps_tsp[:, :LW])
                nc.vector.scalar_tensor_tensor(
                    r_pt[:, tl:th_], tsp[:, tl:th_, 0], 128.0,
                    tsp[:, tl:th_, 1], op0=AluOp.mult, op1=AluOp.add)
                gtmp = wpool.tile([128, ntau, LW], F32, tag=f"gtmp{half}")
                nc.vector.tensor_tensor(gtmp[:], tsp[:, tl:th_],
                                        oh_te_f[:, tl:th_], op=AluOp.mult)
                nc.vector.tensor_reduce(g_wr[:, tl:th_], gtmp[:], axis=AXL.X,
                                        op=AluOp.add)
                idx16 = wpool.tile([16, 8 * ntau], F32, tag=f"idx16{half}")
                for g in range(8):
                    ps_g = psapool.tile([16, ntau], F32, tag="ps_big",
                                        name=f"ps_selg{half}_{g}")
                    nc.tensor.matmul(ps_g[:], lhsT=sel8_sb[:, g],
                                     rhs=r_pt[:, tl:th_],
                                     start=True, stop=True)
                    nc.vector.tensor_copy(idx16[:, g::8], ps_g[:])
                ps_rep = psapool.tile([128, 8 * ntau], F32, tag="ps_big",
                                      name=f"ps_rep{half}")
                nc.tensor.matmul(ps_rep[:], lhsT=rep16_sb[:], rhs=idx16[:],
                                 start=True, stop=True)
                nc.vector.tensor_copy(idxw[:, tl * 8:th_ * 8], ps_rep[:])

                # gathers for this half's slots
                for s in ([0] if half == 0 else [1, 2, 3, 4]):
                    nt = SLOT_CAP[s]
                    t0 = SLOT_TILES[s][0]
                    xt = gtpool.tile([128, 8, nt * 128], BF16, tag=f"xts{s}",
                                     name=f"xt_s{s}")
                    nc.gpsimd.dma_gather(
                        out_ap=xt[:], in_ap=hidden_bf[:],
                        idxs_ap=idxw[:, t0 * 8:(t0 + nt) * 8],
                        num_idxs=nt * 128, num_idxs_reg=nt * 128,
                        elem_size=H, transpose=True)
                    xts[s] = xt

            # zero-path combine after all gathers are queued
            nc.gpsimd.dma_scatter_add(
                out_ap=partial[:], in_ap=yz[:], idxs_ap=seq_sb[:],
                num_idxs=256, num_idxs_reg=256, elem_size=H)

            # =============== phase 4: FFN + combine ===============
            for s in range(NSL):
                    nt = SLOT_CAP[s]
                    xt = xts[s]
                    if s < 2:
                        w13_sb, w2_sb = wpre[s]
                    else:
                        w13_sb = wlpool.tile([128, 8, 2 * I], BF16,
                                             tag="w13", name=f"w13l_s{s}")
                        w2_sb = wlpool.tile([128, 4, H], BF16, tag="w2",
                                            name=f"w2l_s{s}")
                        nc.scalar.dma_start(w13_sb[:], w13s[s])
                        nc.scalar.dma_start(w2_sb[:], w2s[s])
                    groups = [SLOT_TILES[s][i:i + 2]
                              for i in range(0, nt, 2)]
                    for gi, grp in enumerate(groups):
                        gn = len(grp)
                        gt0 = grp[0]
                        yv = wpool.tile([128, gn, H], BF16, tag=f"yv{gn}",
                                        name=f"yv_s{s}g{gi}")
                        for ti, tau in enumerate(grp):
                            xti = SLOT_TILES[s].index(tau)
                            ps_gu = psapool.tile([128, 2 * I], F32,
                                                 tag="ps_big",
                                                 name=f"ps_gu{tau}")
                            for k in range(8):
                                for n in range(2):
                                    nc.tensor.matmul(
                                        ps_gu[:, n * 512:(n + 1) * 512],
                                        lhsT=xt[:, k,
                                                xti * 128:(xti + 1) * 128],
                                        rhs=w13_sb[:, k,
                                                   n * 512:(n + 1) * 512],
                                        start=(k == 0), stop=(k == 7))
                            sl = wpool.tile([128, I], F32, tag="sl")
                            nc.scalar.activation(sl[:], ps_gu[:, :I],
                                                 ACT_F.Silu)
                            hh = wpool.tile([128, I], F32, tag="hh")
                            nc.vector.tensor_mul(hh[:], sl[:], ps_gu[:, I:])
                            hT = wpool.tile([128, 4, 128], BF16, tag="hT")
                            for k in range(4):
                                ps_t2 = pspool.tile([128, 128], F32,
                                                    tag="ps_tr")
                                nc.tensor.transpose(
                                    ps_t2[:], hh[:, k * 128:(k + 1) * 128],
                                    ident_sb[:])
                                if k % 2 == 0:
                                    nc.vector.tensor_copy(hT[:, k], ps_t2[:])
                                else:
                                    nc.scalar.activation(hT[:, k], ps_t2[:],
                                                         ACT_F.Copy)
                            ps_y = psapool.tile([128, H], F32, tag="ps_big",
                                                name=f"ps_y{tau}")
                            for k in range(4):
                                for n in range(2):
                                    nc.tensor.matmul(
                                        ps_y[:, n * 512:(n + 1) * 512],
                                        lhsT=hT[:, k],
                                        rhs=w2_sb[:, k,
                                                  n * 512:(n + 1) * 512],
                                        start=(k == 0), stop=(k == 3))
                            nc.vector.tensor_scalar(yv[:, ti, :], ps_y[:],
                                                    g_wr[:, tau:tau + 1],
                                                    None, op0=AluOp.mult)
                        nc.gpsimd.dma_scatter_add(
                            out_ap=partial[:], in_ap=yv[:],
                            idxs_ap=idxw[:, gt0 * 8:(gt0 + gn) * 8],
                            num_idxs=gn * 128, num_idxs_reg=gn * 128,
                            elem_size=H)

    nc.compile()
    _NC_CACHE[key] = nc
    return nc


# ---------------------------------------------------------------------------
# host wrapper
# ---------------------------------------------------------------------------

def make_in_maps(hidden_states, router_w, e_score_correction_bias, w13, w2):
    import ml_dtypes
    hidden_states = np.asarray(hidden_states, np.float32)
    router_w = np.asarray(router_w, np.float32)
    bias = np.asarray(e_score_correction_bias, np.float32)
    w13 = np.asarray(w13, np.float32)
    w2 = np.asarray(w2, np.float32)

    ids = _host_routing(hidden_states, router_w, bias)
    slot_expert, tiles = _schedule(ids)

    # r = (t % 128) * 16 + t // 128  <->  t = (r % 16) * 128 + r // 16
    r_of_t = (np.arange(T) % 128) * 16 + np.arange(T) // 128
    t_of_r = np.empty(T, np.int64)
    t_of_r[r_of_t] = np.arange(T)

    hidden_T = np.ascontiguousarray(hidden_states.T)
    # [k, p, cq, u] -> [cq, p, k, u]
    hidden_Tt = np.ascontiguousarray(
        hidden_T.reshape(8, 128, 8, 256).transpose(2, 1, 0, 3))
    hidden_rows = np.ascontiguousarray(hidden_states[t_of_r])
    hidden_bf = hidden_rows.astype(ml_dtypes.bfloat16)
    rwt = np.ascontiguousarray(router_w.T)
    bias_b = np.tile(bias[None, :], (128, 1))
    w13t = w13.transpose(0, 2, 1)                  # [e, h, 2I]
    w2t = w2.transpose(0, 2, 1)                    # [e, i, h]
    # host-tiled contiguous weight layout: [e, p, k, i]
    w13tt = np.ascontiguousarray(
        w13t.reshape(ER, 8, 128, 2 * I).transpose(0, 2, 1, 3)).astype(
            ml_dtypes.bfloat16)
    w2tt = np.ascontiguousarray(
        w2t.reshape(ER, 4, 128, H).transpose(0, 2, 1, 3)).astype(
            ml_dtypes.bfloat16)

    rr = np.arange(T).reshape(128, NJ).astype(np.float32)  # r at [p, j]
    rhl = np.stack([rr // 128, rr % 128], axis=-1).astype(ml_dtypes.bfloat16)
    iota42m2 = np.tile(np.arange(-2, NE, dtype=np.float32), (128, TMAX, 1))
    iota128r = np.tile(np.arange(128, dtype=np.float32), (128, TMAX, 1)) \
        .astype(ml_dtypes.bfloat16)
    ident = np.eye(128, dtype=np.float32)
    uts128 = np.triu(np.ones((128, 128), np.float32), k=1)
    rep16 = np.zeros((16, 128), np.float32)
    rep16[np.arange(128) % 16, np.arange(128)] = 1.0
    sel8 = np.zeros((128, 8, 16), np.float32)
    for g in range(8):
        sel8[16 * g + np.arange(16), g, np.arange(16)] = 1.0

    p_ = np.arange(128)[:, None]
    f_ = np.arange(16)[None, :]
    seq_base = (f_ % 8) * 16 + (p_ % 16) + (f_ // 8) * 128  # [p, f]

    in_maps = []
    for c in range(NCORES):
        te = np.array([tiles[c][tau][0] for tau in range(TMAX)], np.float32)
        tlo = np.array([tiles[c][tau][1] for tau in range(TMAX)], np.float32)
        in_maps.append({
            "hidden_Tt": hidden_Tt,
            "hidden_bf": hidden_bf,
            "rwt": rwt,
            "bias_b": bias_b,
            "w13s": np.ascontiguousarray(
                w13tt[[slot_expert[c][s] for s in range(NSL)]]),
            "w2s": np.ascontiguousarray(
                w2tt[[slot_expert[c][s] for s in range(NSL)]]),
            "tile_e": np.tile(te[None, :], (128, 1)),
            "tile_lo": np.tile(tlo[None, :], (128, 1)),
            "rhl": rhl,
            "iota42m2": iota42m2,
            "iota128r": iota128r,
            "ident": ident,
            "uts128": uts128,
            "rep16": rep16,
            "sel8": sel8,
            "hz": np.ascontiguousarray(hidden_rows[c * 256:(c + 1) * 256]),
            "seqidx": (seq_base + c * 256).astype(np.int16),
        })
    return in_maps, t_of_r


def kernel(hidden_states, router_w, e_score_correction_bias, w13, w2,
           _trace=False):
    nc = build_nc()
    in_maps, t_of_r = make_in_maps(hidden_states, router_w,
                                   e_score_correction_bias, w13, w2)
    res = run_bass_kernel_spmd(nc, in_maps, core_ids=list(range(NCORES)),
                               trace=_trace)
    total = np.zeros((T, H), np.float64)
    for c in range(NCORES):
        total += res.results[c]["partial"].astype(np.float64)
    out = np.empty((T, H), np.float32)
    out[t_of_r] = total.astype(np.float32)      # out[t] = total[r(t)]
    kernel._last_results = res
    return out


# revision 12
# speedup vs baseline: 1.7351x; 1.1237x over previous
"""LongcatFlash MoE kernel for 8 TRN2 NeuronCores (expert-parallel).

Contract: kernel(**inputs) takes the FULL un-sharded inputs from
reference.setup_inputs() and returns the FULL [T, H] output.

Strategy (expert-parallel, memory-regime), v4:
  - Router replicated, exact fp32 matmuls; PE pre-warmed with dummy
    matmuls so the stream runs at full clock. Softmax / top-4-threshold
    / mask / per-tile-membership vector work is chunked per 512 tokens
    and runs under the router matmul stream.
  - Selection is a mask (biased >= 4th-largest); gates flow as the
    masked-scaled score vector straight into the inversion matmul.
  - Dispatch inversion is split into two tau-groups: slot-0's tiles
    decode + gather + compute while the second group still inverts.
  - Slot->token lists built on-chip with permutation matmuls.
  - FFN: bf16 weights (DMA gated behind router start so the hidden
    chunks win early bandwidth), transposed bf16 gather for x, Silu on
    the scalar engine, bf16 partial output with CCE scatter-add.
  - Zero-experts (ids >= 32) reduce to a per-token scale of the hidden
    row, applied by the token-range owner core.
  - Host unshards by summing the 8 partial outputs and undoing the row
    permutation r = (t % 128) * 16 + t // 128.
"""

import numpy as np

import concourse.bacc as bacc
import concourse.bass as bass
import concourse.mybir as mybir
import concourse.tile as tile
from concourse import library_config
from concourse.bass_utils import run_bass_kernel_spmd

F32 = mybir.dt.float32
BF16 = mybir.dt.bfloat16
I16 = mybir.dt.int16
U8 = mybir.dt.uint8

T, H, I = 2048, 1024, 512
NE, ER = 40, 32
ROUTED_SCALE = 2.5
NCORES = 8
NJ = T // 128              # 16 token tiles (r = p*16 + j)
TMAX = 9                   # static FFN tiles per core
NSL = 5                    # weight slots per core
SLOT_CAP = [4, 2, 1, 1, 1]
SLOT_TILES = [[0, 1, 2, 3], [4, 5], [6], [7], [8]]
NSLOT = TMAX * 128         # 1152 dispatch slots per core
LW = 2 + NE                # inversion lhsT width: r_hi, r_lo, 40 gates
NTA = 4                    # tau-group A: taus 0..3 (slot 0)
NTB = TMAX - NTA           # tau-group B: taus 4..8 (slots 1..4)
AluOp = mybir.AluOpType
ACT_F = mybir.ActivationFunctionType
AXL = mybir.AxisListType


# ---------------------------------------------------------------------------
# host-side schedule
# ---------------------------------------------------------------------------

def _host_routing(hidden, router_w, bias):
    """fp32 routing on host — used ONLY for load-balance scheduling."""
    logits = hidden.astype(np.float32) @ router_w.astype(np.float32).T
    m = logits.max(axis=1, keepdims=True)
    e = np.exp(logits - m)
    scores = e / e.sum(axis=1, keepdims=True)
    biased = scores + bias[None, :]
    ids = np.argsort(-biased, axis=1, kind="stable")[:, :4]
    return ids


def _schedule(ids):
    """Static tile schedule: split-anywhere first-fit-decreasing packing.

    Returns per-core:
      slot_expert[c][s]: global expert id serviced by local weight slot s
      tiles[c][tau]: (expert_id, lo_rank) — dispatch range for FFN tile tau
    Ranks are positions within an expert's selected-token list in r-order.
    """
    counts = np.zeros(ER, np.int64)
    for row in ids:
        for e in row:
            if e < ER:
                counts[e] += 1
    pieces = [[e, 0, (int(counts[e]) + 127) // 128] for e in range(ER)
              if counts[e] > 0]               # [expert, first_tile, ntiles]
    pieces.sort(key=lambda p: -p[2])
    slots = sorted(((SLOT_CAP[s], c, s) for c in range(NCORES)
                    for s in range(NSL)), key=lambda x: -x[0])
    slot_expert = [[0] * NSL for _ in range(NCORES)]
    tiles = [[(0, 1 << 14)] * TMAX for _ in range(NCORES)]
    si = 0
    work = []
    for p in pieces:
        work.append(p)
    while work:
        work.sort(key=lambda p: -p[2])
        p = work.pop(0)
        if si >= len(slots):
            raise RuntimeError("schedule: out of weight slots")
        cap, c, s = slots[si]
        si += 1
        take = min(cap, p[2])
        slot_expert[c][s] = p[0]
        for k in range(cap):
            tau = SLOT_TILES[s][k]
            # tiles beyond `take` extend the range as harmless slack
            tiles[c][tau] = (p[0], 128 * (p[1] + min(k, take)))
        for k in range(take):
            tiles[c][SLOT_TILES[s][k]] = (p[0], 128 * (p[1] + k))
        if p[2] > take:
            work.append([p[0], p[1] + take, p[2] - take])
    return slot_expert, tiles


# ---------------------------------------------------------------------------
# device graph
# ---------------------------------------------------------------------------

_NC_CACHE = {}


def build_nc():
    key = "v4"
    if key in _NC_CACHE:
        return _NC_CACHE[key]
    nc = bacc.Bacc("TRN2", target_bir_lowering=False, debug=False,
                   num_devices=NCORES)

    def din(name, shape, dt):
        return nc.dram_tensor(name, shape, dt, kind="ExternalInput").ap()

    hidden_Tt = din("hidden_Tt", [8, 128, 8, 256], F32)  # router chunks
    hidden_bf = din("hidden_bf", [T, H], BF16)         # r-ordered rows, bf16
    rwt = din("rwt", [H, NE], F32)                     # router_w.T
    bias_b = din("bias_b", [128, NE], F32)             # bias replicated
    w13s = din("w13s", [NSL, 128, 8, 2 * I], BF16)     # [slot, p, k, 2i]
    w2s = din("w2s", [NSL, 128, 4, H], BF16)           # [slot, p, k, h]
    tile_e = din("tile_e", [128, TMAX], F32)           # expert id per tile
    tile_lo = din("tile_lo", [128, TMAX], F32)         # rank range lo per tile
    rhl = din("rhl", [128, NJ, 2], BF16)               # r split (r//128, r%128)
    iota42m2 = din("iota42m2", [128, TMAX, LW], F32)   # value = col - 2
    iota128r = din("iota128r", [128, TMAX, 128], BF16) # value = col (0..127)
    ident = din("ident", [128, 128], F32)
    uts128 = din("uts128", [128, 128], F32)            # strict upper: [k,m]=k<m
    rep16 = din("rep16", [16, 128], F32)               # rep16[q,p] = (p%16==q)
    sel8 = din("sel8", [128, 8, 16], F32)              # sel8[p,g,q] = (p==16g+q)
    hz = din("hz", [256, H], F32)                      # my zero-path rows
    seqidx = din("seqidx", [128, 16], I16)             # my zero-path idxs

    partial = nc.dram_tensor("partial", [T, H], BF16,
                             kind="ExternalOutput").ap()

    with tile.TileContext(nc) as tc:
        with (
            tc.tile_pool(name="const", bufs=1) as cpool,
            tc.tile_pool(name="work", bufs=2) as wpool,
            tc.tile_pool(name="persist", bufs=1) as ppool,
            tc.tile_pool(name="wload", bufs=2) as wlpool,
            tc.tile_pool(name="hts", bufs=3) as htpool,
            tc.tile_pool(name="gt", bufs=1) as gtpool,
            tc.tile_pool(name="psum", bufs=2, space="PSUM") as pspool,
            tc.tile_pool(name="psumA", bufs=3, space="PSUM") as psapool,
            tc.tile_pool(name="dram", bufs=1, space="DRAM") as dpool,
        ):
            nc.gpsimd.load_library(library_config.mlp)

            # ---- router weights + token chunks first on the sync queue ----
            rw_sb = cpool.tile([128, 8, NE], F32, tag="rw")
            nc.sync.dma_start(rw_sb[:],
                              rwt.rearrange("(k p) n -> p k n", p=128))
            ident_sb = cpool.tile([128, 128], F32, tag="ident")
            nc.sync.dma_start(ident_sb[:], ident[:])
            hts = []
            for cq in range(8):
                ht = htpool.tile([128, 8, 256], F32, tag="ht",
                                 name=f"ht{cq}")
                nc.sync.dma_start(ht[:], hidden_Tt[cq])
                hts.append(ht)

            # ---- resident constants (gpsimd DMA queue) ----
            bias_sb = cpool.tile([128, NE], F32, tag="bias")
            nc.gpsimd.dma_start(bias_sb[:], bias_b[:])
            uts_sb = cpool.tile([128, 128], F32, tag="uts")
            nc.gpsimd.dma_start(uts_sb[:], uts128[:])
            te_sb = cpool.tile([128, TMAX], F32, tag="te")
            nc.gpsimd.dma_start(te_sb[:], tile_e[:])
            tlo_sb = cpool.tile([128, TMAX], F32, tag="tlo")
            nc.gpsimd.dma_start(tlo_sb[:], tile_lo[:])
            i42_sb = cpool.tile([128, TMAX, LW], F32, tag="i42")
            nc.gpsimd.dma_start(i42_sb[:], iota42m2[:])
            i128_sb = cpool.tile([128, TMAX, 128], BF16, tag="i128")
            nc.gpsimd.dma_start(i128_sb[:], iota128r[:])
            rep16_sb = cpool.tile([16, 128], F32, tag="rep16")
            nc.gpsimd.dma_start(rep16_sb[:], rep16[:])
            sel8_sb = cpool.tile([128, 8, 16], F32, tag="sel8")
            nc.gpsimd.dma_start(sel8_sb[:], sel8[:])
            seq_sb = cpool.tile([128, 16], I16, tag="seqsb")
            nc.gpsimd.dma_start(seq_sb[:], seqidx[:])
            zeros16 = cpool.tile([128, NJ], F32, tag="z16")
            nc.vector.memset(zeros16[:], 0.0)

            # expert one-hot per tile: oh_te_f[p, tau, 2+e] = (e == te[tau])
            oh_te_f = cpool.tile([128, TMAX, LW], F32, tag="ohtef")
            nc.vector.tensor_tensor(
                oh_te_f[:], i42_sb[:],
                te_sb[:].unsqueeze(2).to_broadcast([128, TMAX, LW]),
                op=AluOp.is_equal)
            oh_te_b = cpool.tile([128, TMAX, LW], BF16, tag="ohteb")
            nc.vector.tensor_copy(oh_te_b[:], oh_te_f[:])

            # ---- persistent intermediates ----
            lhsT_all = ppool.tile([128, NJ, LW], BF16, tag="lhsT")
            nc.gpsimd.dma_start(lhsT_all[:, :, 0:2], rhl[:])
            sc3 = ppool.tile([128, NJ, NE], F32, tag="sc3")
            scores = ppool.tile([128, NJ, NE], F32, tag="scores")
            biased = ppool.tile([128, NJ, NE], F32, tag="biased")
            thr = ppool.tile([128, NJ, 1], F32, tag="thr")
            mask3 = ppool.tile([128, NJ, NE], F32, tag="mask3")
            mask3b = ppool.tile([128, NJ, NE], BF16, tag="mask3b")
            mg = ppool.tile([128, NJ, NE], F32, tag="mg")
            zt_all = ppool.tile([128, NJ], F32, tag="zt")
            mask_tau = ppool.tile([128, TMAX, NJ], F32, tag="masktau")

            # ---- PE warm-up (HAM) on the router weights ----
            for w in range(24):
                ps_w = pspool.tile([128, 128], F32, tag="ps_tr",
                                   name=f"ps_warm{w}")
                nc.tensor.matmul(ps_w[:NE, :], lhsT=rw_sb[:, 0, :],
                                 rhs=ident_sb[:], start=True, stop=True)

            # =============== phase 1: router (exact fp32) ===============
            wgate = dpool.tile([1, 1], F32, tag="wgate")
            for cq in range(8):
                ht = hts[cq]
                ps_lg = psapool.tile([40, 256], F32, tag="ps_big",
                                     name=f"ps_lg{cq}")
                for k in range(8):
                    nc.tensor.matmul(ps_lg[:], lhsT=rw_sb[:, k, :],
                                     rhs=ht[:, k, :],
                                     start=(k == 0), stop=(k == 7))
                lgs = wpool.tile([40, 256], F32, tag="lgs")
                nc.vector.tensor_copy(lgs[:], ps_lg[:])
                for q in range(2):
                    j = cq * 2 + q
                    ps_l = pspool.tile([128, 128], F32, tag="ps_tr",
                                       name=f"ps_lt{j}")
                    nc.tensor.transpose(ps_l[:, :NE],
                                        lgs[:, q * 128:(q + 1) * 128],
                                        ident_sb[:NE, :NE])
                    nc.vector.tensor_copy(sc3[:, j, :], ps_l[:, :NE])
                if cq == 6:
                    # gate the weight stream until most hidden chunks landed
                    nc.scalar.dma_start(wgate[:], sc3[0:1, 12, 0:1])
                    wpre = []
                    for s in range(2):
                        w13_sb = wlpool.tile([128, 8, 2 * I], BF16, tag="w13",
                                             name=f"w13_s{s}")
                        w2_sb = wlpool.tile([128, 4, H], BF16, tag="w2",
                                            name=f"w2_s{s}")
                        nc.scalar.dma_start(w13_sb[:], w13s[s])
                        nc.scalar.dma_start(w2_sb[:], w2s[s])
                        wpre.append((w13_sb, w2_sb))
                if cq % 2 == 0:
                    continue
                g = cq // 2
                js = slice(g * 4, g * 4 + 4)
                sh = [128, 4, NE]
                rmax = wpool.tile([128, 4, 1], F32, tag="rmax")
                nc.vector.tensor_reduce(rmax[:], sc3[:, js], axis=AXL.X,
                                        op=AluOp.max, negate=True)  # -max
                xs = wpool.tile(sh, F32, tag="xs")
                nc.vector.tensor_tensor(xs[:], sc3[:, js],
                                        rmax[:].to_broadcast(sh),
                                        op=AluOp.add)
                exv = wpool.tile(sh, F32, tag="ex")
                nc.scalar.activation(exv[:], xs[:], ACT_F.Exp)
                rsum = wpool.tile([128, 4, 1], F32, tag="rsum")
                nc.vector.tensor_reduce(rsum[:], exv[:], axis=AXL.X,
                                        op=AluOp.add)
                rinv = wpool.tile([128, 4, 1], F32, tag="rinv")
                nc.vector.reciprocal(rinv[:], rsum[:])
                nc.vector.tensor_tensor(scores[:, js], exv[:],
                                        rinv[:].to_broadcast(sh),
                                        op=AluOp.mult)
                nc.vector.tensor_tensor(
                    biased[:, js], scores[:, js],
                    bias_sb[:].unsqueeze(1).to_broadcast(sh),
                    op=AluOp.add)
                for j in range(g * 4, g * 4 + 4):
                    t8 = wpool.tile([128, 8], F32, tag="t8")
                    nc.vector.max(t8[:], biased[:, j])
                    nc.vector.tensor_copy(thr[:, j, :], t8[:, 3:4])
                nc.vector.tensor_tensor(mask3[:, js], biased[:, js],
                                        thr[:, js].to_broadcast(sh),
                                        op=AluOp.is_ge)
                nc.vector.scalar_tensor_tensor(mg[:, js], mask3[:, js],
                                               ROUTED_SCALE, scores[:, js],
                                               op0=AluOp.mult, op1=AluOp.mult)
                nc.vector.tensor_reduce(zt_all[:, js], mg[:, js, ER:NE],
                                        axis=AXL.X, op=AluOp.add)
                nc.vector.tensor_copy(lhsT_all[:, js, 2:LW], mg[:, js])
                nc.vector.tensor_copy(mask3b[:, js], mask3[:, js])
                # tile membership for this 4-j group (dual broadcast)
                mtmp = wpool.tile([128, TMAX, 4, NE], BF16, tag="mtmp")
                nc.vector.tensor_tensor(
                    mtmp[:],
                    mask3b[:, js].unsqueeze(1).to_broadcast(
                        [128, TMAX, 4, NE]),
                    oh_te_b[:, :, 2:LW].unsqueeze(2).to_broadcast(
                        [128, TMAX, 4, NE]),
                    op=AluOp.mult)
                nc.vector.tensor_reduce(mask_tau[:, :, js], mtmp[:],
                                        axis=AXL.X, op=AluOp.add)

            # =============== zero-expert path (scatter emitted later) ====
            hzts = []
            for tt in range(2):
                hzt = wpool.tile([128, H], F32, tag="hzt")
                nc.sync.dma_start(hzt[:], hz[tt * 128:(tt + 1) * 128, :])
                hzts.append(hzt)
            zt_flat = dpool.tile([1, T], F32, tag="ztflat")
            nc.sync.dma_start(zt_flat[0, :].rearrange("(p j) -> p j", p=128),
                              zt_all[:])
            pid = nc.sync.partition_id()
            yz = ppool.tile([128, 2, H], BF16, tag="yz")
            for tt in range(2):
                ztv = wpool.tile([1, 128], F32, tag="ztv")
                nc.sync.dma_start(
                    ztv[:], zt_flat[0:1, bass.ds(pid * 256 + tt * 128, 128)])
                ps_zt = psapool.tile([128, 1], F32, tag="ps_big",
                                     name=f"ps_zt{tt}")
                nc.tensor.transpose(ps_zt[:], ztv[:], ident_sb[:1, :1])
                ztc = wpool.tile([128, 1], F32, tag="ztc")
                nc.vector.tensor_copy(ztc[:], ps_zt[:])
                nc.scalar.activation(yz[:, tt, :], hzts[tt][:], ACT_F.Copy,
                                     scale=ztc[:, 0:1])

            # =============== phase 2: ranks ===============
            inrow = wpool.tile([128, TMAX, NJ], F32, tag="inrow")
            for tau in range(TMAX):
                nc.vector.tensor_tensor_scan(inrow[:, tau], mask_tau[:, tau],
                                             zeros16[:], 0.0,
                                             op0=AluOp.add, op1=AluOp.add)
            rowsum = wpool.tile([128, TMAX], F32, tag="rowsum")
            nc.vector.tensor_reduce(rowsum[:], mask_tau[:], axis=AXL.X,
                                    op=AluOp.add)
            ps_rp = psapool.tile([128, TMAX], F32, tag="ps_big",
                                 name="ps_rp")
            nc.tensor.matmul(ps_rp[:], lhsT=uts_sb[:], rhs=rowsum[:],
                             start=True, stop=True)
            pref = wpool.tile([128, TMAX, 1], F32, tag="pref")
            nc.vector.tensor_copy(pref[:, :, 0], ps_rp[:])
            pos = wpool.tile([128, TMAX, NJ], F32, tag="pos")
            nc.vector.tensor_tensor(pos[:], inrow[:],
                                    pref[:].to_broadcast([128, TMAX, NJ]),
                                    op=AluOp.add)
            nc.vector.tensor_sub(pos[:], pos[:], mask_tau[:])
            t1 = wpool.tile([128, TMAX, NJ], F32, tag="t1")
            nc.vector.tensor_tensor(
                t1[:], pos[:],
                tlo_sb[:].unsqueeze(2).to_broadcast([128, TMAX, NJ]),
                op=AluOp.subtract)
            okr = wpool.tile([128, TMAX, NJ], F32, tag="okr")
            nc.vector.tensor_scalar(okr[:], t1[:], -0.5, None, op0=AluOp.is_gt)
            ok2 = wpool.tile([128, TMAX, NJ], F32, tag="ok2")
            nc.vector.tensor_scalar(ok2[:], t1[:], 127.5, None, op0=AluOp.is_lt)
            nc.vector.tensor_mul(okr[:], okr[:], ok2[:])
            nc.vector.tensor_mul(okr[:], okr[:], mask_tau[:])
            oku = wpool.tile([128, TMAX, NJ], U8, tag="oku")
            nc.vector.tensor_copy(oku[:], okr[:])
            t1m = wpool.tile([128, TMAX, NJ], F32, tag="t1m")
            nc.vector.memset(t1m[:], -4.0)
            nc.vector.copy_predicated(t1m[:], oku[:], t1[:])
            t1mb = ppool.tile([128, TMAX, NJ], BF16, tag="t1mb")
            nc.vector.tensor_copy(t1mb[:], t1m[:])

            # =============== phase 3: split inversion + decode ===============
            tsp = ppool.tile([128, TMAX, LW], F32, tag="tsp")
            r_pt = ppool.tile([128, TMAX], F32, tag="rpt")
            g_wr = ppool.tile([128, TMAX], F32, tag="gwr")
            idxw = ppool.tile([128, 8 * TMAX], I16, tag="idxw")
            xts = {}

            for half, (tl, th_) in enumerate([(0, NTA), (NTA, TMAX)]):
                ntau = th_ - tl
                inv_ps = psapool.tile([LW, ntau * 128], F32, tag="ps_big",
                                      name=f"ps_inv{half}")
                for j in range(NJ):
                    oh = wpool.tile([128, ntau, 128], BF16, tag=f"oh{half}")
                    nc.vector.tensor_tensor(
                        oh[:],
                        t1mb[:, tl:th_, j].unsqueeze(2).to_broadcast(
                            [128, ntau, 128]),
                        i128_sb[:, tl:th_, :],
                        op=AluOp.is_equal)
                    ohf = oh[:].rearrange("p a b -> p (a b)")
                    for lo in range(0, ntau * 128, 512):
                        hi = min(lo + 512, ntau * 128)
                        nc.tensor.matmul(inv_ps[:, lo:hi],
                                         lhsT=lhsT_all[:, j],
                                         rhs=ohf[:, lo:hi],
                                         start=(j == 0), stop=(j == NJ - 1))
                inv_sb = ppool.tile([LW, ntau * 128], F32, tag=f"invsb{half}")
                nc.vector.tensor_copy(inv_sb[:], inv_ps[:])
                for ti in range(ntau):
                    tau = tl + ti
                    ps_tsp = pspool.tile([128, 128], F32, tag="ps_tr",
                                         name=f"ps_tsp{tau}")
                    nc.tensor.transpose(ps_tsp[:, :LW],
                                        inv_sb[:, ti * 128:(ti + 1) * 128],
                                        ident_sb[:LW, :LW])
                    nc.vector.tensor_copy(tsp[:, tau], ps_tsp[:, :LW])
                nc.vector.scalar_tensor_tensor(
                    r_pt[:, tl:th_], tsp[:, tl:th_, 0], 128.0,
                    tsp[:, tl:th_, 1], op0=AluOp.mult, op1=AluOp.add)
                gtmp = wpool.tile([128, ntau, LW], F32, tag=f"gtmp{half}")
                nc.vector.tensor_tensor(gtmp[:], tsp[:, tl:th_],
                                        oh_te_f[:, tl:th_], op=AluOp.mult)
                nc.vector.tensor_reduce(g_wr[:, tl:th_], gtmp[:], axis=AXL.X,
                                        op=AluOp.add)
                idx16 = wpool.tile([16, 8 * ntau], F32, tag=f"idx16{half}")
                for g in range(8):
                    ps_g = psapool.tile([16, ntau], F32, tag="ps_big",
                                        name=f"ps_selg{half}_{g}")
                    nc.tensor.matmul(ps_g[:], lhsT=sel8_sb[:, g],
                                     rhs=r_pt[:, tl:th_],
                                     start=True, stop=True)
                    nc.vector.tensor_copy(idx16[:, g::8], ps_g[:])
                ps_rep = psapool.tile([128, 8 * ntau], F32, tag="ps_big",
                                      name=f"ps_rep{half}")
                nc.tensor.matmul(ps_rep[:], lhsT=rep16_sb[:], rhs=idx16[:],
                                 start=True, stop=True)
                nc.vector.tensor_copy(idxw[:, tl * 8:th_ * 8], ps_rep[:])

                # gathers for this half's slots
                for s in ([0] if half == 0 else [1, 2, 3, 4]):
                    nt = SLOT_CAP[s]
                    t0 = SLOT_TILES[s][0]
                    xt = gtpool.tile([128, 8, nt * 128], BF16, tag=f"xts{s}",
                                     name=f"xt_s{s}")
                    nc.gpsimd.dma_gather(
                        out_ap=xt[:], in_ap=hidden_bf[:],
                        idxs_ap=idxw[:, t0 * 8:(t0 + nt) * 8],
                        num_idxs=nt * 128, num_idxs_reg=nt * 128,
                        elem_size=H, transpose=True)
                    xts[s] = xt

            # zero-path combine after all gathers are queued
            nc.gpsimd.dma_scatter_add(
                out_ap=partial[:], in_ap=yz[:], idxs_ap=seq_sb[:],
                num_idxs=256, num_idxs_reg=256, elem_size=H)

            # =============== phase 4: FFN + combine ===============
            for s in range(NSL):
                    nt = SLOT_CAP[s]
                    xt = xts[s]
                    if s < 2:
                        w13_sb, w2_sb = wpre[s]
                    else:
                        w13_sb = wlpool.tile([128, 8, 2 * I], BF16,
                                             tag="w13", name=f"w13l_s{s}")
                        w2_sb = wlpool.tile([128, 4, H], BF16, tag="w2",
                                            name=f"w2l_s{s}")
                        nc.scalar.dma_start(w13_sb[:], w13s[s])
                        nc.scalar.dma_start(w2_sb[:], w2s[s])
                    groups = [SLOT_TILES[s][i:i + 2]
                              for i in range(0, nt, 2)]
                    for gi, grp in enumerate(groups):
                        gn = len(grp)
                        gt0 = grp[0]
                        yv = wpool.tile([128, gn, H], BF16, tag=f"yv{gn}",
                                        name=f"yv_s{s}g{gi}")
                        for ti, tau in enumerate(grp):
                            xti = SLOT_TILES[s].index(tau)
                            ps_gu = psapool.tile([128, 2 * I], F32,
                                                 tag="ps_big",
                                                 name=f"ps_gu{tau}")
                            for n in range(2):
                                for k in range(8):
                                    nc.tensor.matmul(
                                        ps_gu[:, n * 512:(n + 1) * 512],
                                        lhsT=xt[:, k,
                                                xti * 128:(xti + 1) * 128],
                                        rhs=w13_sb[:, k,
                                                   n * 512:(n + 1) * 512],
                                        start=(k == 0), stop=(k == 7))
                            sl = wpool.tile([128, I], F32, tag="sl")
                            nc.scalar.activation(sl[:], ps_gu[:, :I],
                                                 ACT_F.Silu)
                            hh = wpool.tile([128, I], F32, tag="hh")
                            nc.vector.tensor_mul(hh[:], sl[:], ps_gu[:, I:])
                            hT = wpool.tile([128, 4, 128], BF16, tag="hT")
                            for k in range(4):
                                ps_t2 = pspool.tile([128, 128], F32,
                                                    tag="ps_tr")
                                nc.tensor.transpose(
                                    ps_t2[:], hh[:, k * 128:(k + 1) * 128],
                                    ident_sb[:])
                                if k % 2 == 0:
                                    nc.vector.tensor_copy(hT[:, k], ps_t2[:])
                                else:
                                    nc.scalar.activation(hT[:, k], ps_t2[:],
                                                         ACT_F.Copy)
                            ps_y = psapool.tile([128, H], F32, tag="ps_big",
                                                name=f"ps_y{tau}")
                            for k in range(4):
                                for n in range(2):
                                    nc.tensor.matmul(
                                        ps_y[:, n * 512:(n + 1) * 512],
                                        lhsT=hT[:, k],
                                        rhs=w2_sb[:, k,
                                                  n * 512:(n + 1) * 512],
                                        start=(k == 0), stop=(k == 3))
                            nc.vector.tensor_scalar(yv[:, ti, :I], ps_y[:, :I],
                                                    g_wr[:, tau:tau + 1],
                                                    None, op0=AluOp.mult)
                            nc.scalar.activation(yv[:, ti, I:], ps_y[:, I:],
                                                 ACT_F.Copy,
                                                 scale=g_wr[:, tau:tau + 1])
                        nc.gpsimd.dma_scatter_add(
                            out_ap=partial[:], in_ap=yv[:],
                            idxs_ap=idxw[:, gt0 * 8:(gt0 + gn) * 8],
                            num_idxs=gn * 128, num_idxs_reg=gn * 128,
                            elem_size=H)

    nc.compile()
    _NC_CACHE[key] = nc
    return nc


# ---------------------------------------------------------------------------
# host wrapper
# ---------------------------------------------------------------------------

def make_in_maps(hidden_states, router_w, e_score_correction_bias, w13, w2):
    import ml_dtypes
    hidden_states = np.asarray(hidden_states, np.float32)
    router_w = np.asarray(router_w, np.float32)
    bias = np.asarray(e_score_correction_bias, np.float32)
    w13 = np.asarray(w13, np.float32)
    w2 = np.asarray(w2, np.float32)

    ids = _host_routing(hidden_states, router_w, bias)
    slot_expert, tiles = _schedule(ids)

    # r = (t % 128) * 16 + t // 128  <->  t = (r % 16) * 128 + r // 16
    r_of_t = (np.arange(T) % 128) * 16 + np.arange(T) // 128
    t_of_r = np.empty(T, np.int64)
    t_of_r[r_of_t] = np.arange(T)

    hidden_T = np.ascontiguousarray(hidden_states.T)
    # [k, p, cq, u] -> [cq, p, k, u]
    hidden_Tt = np.ascontiguousarray(
        hidden_T.reshape(8, 128, 8, 256).transpose(2, 1, 0, 3))
    hidden_rows = np.ascontiguousarray(hidden_states[t_of_r])
    hidden_bf = hidden_rows.astype(ml_dtypes.bfloat16)
    rwt = np.ascontiguousarray(router_w.T)
    bias_b = np.tile(bias[None, :], (128, 1))
    w13t = w13.transpose(0, 2, 1)                  # [e, h, 2I]
    w2t = w2.transpose(0, 2, 1)                    # [e, i, h]
    # host-tiled contiguous weight layout: [e, p, k, i]
    w13tt = np.ascontiguousarray(
        w13t.reshape(ER, 8, 128, 2 * I).transpose(0, 2, 1, 3)).astype(
            ml_dtypes.bfloat16)
    w2tt = np.ascontiguousarray(
        w2t.reshape(ER, 4, 128, H).transpose(0, 2, 1, 3)).astype(
            ml_dtypes.bfloat16)

    rr = np.arange(T).reshape(128, NJ).astype(np.float32)  # r at [p, j]
    rhl = np.stack([rr // 128, rr % 128], axis=-1).astype(ml_dtypes.bfloat16)
    iota42m2 = np.tile(np.arange(-2, NE, dtype=np.float32), (128, TMAX, 1))
    iota128r = np.tile(np.arange(128, dtype=np.float32), (128, TMAX, 1)) \
        .astype(ml_dtypes.bfloat16)
    ident = np.eye(128, dtype=np.float32)
    uts128 = np.triu(np.ones((128, 128), np.float32), k=1)
    rep16 = np.zeros((16, 128), np.float32)
    rep16[np.arange(128) % 16, np.arange(128)] = 1.0
    sel8 = np.zeros((128, 8, 16), np.float32)
    for g in range(8):
        sel8[16 * g + np.arange(16), g, np.arange(16)] = 1.0

    p_ = np.arange(128)[:, None]
    f_ = np.arange(16)[None, :]
    seq_base = (f_ % 8) * 16 + (p_ % 16) + (f_ // 8) * 128  # [p, f]

    in_maps = []
    for c in range(NCORES):
        te = np.array([tiles[c][tau][0] for tau in range(TMAX)], np.float32)
        tlo = np.array([tiles[c][tau][1] for tau in range(TMAX)], np.float32)
        in_maps.append({
            "hidden_Tt": hidden_Tt,
            "hidden_bf": hidden_bf,
            "rwt": rwt,
            "bias_b": bias_b,
            "w13s": np.ascontiguousarray(
                w13tt[[slot_expert[c][s] for s in range(NSL)]]),
            "w2s": np.ascontiguousarray(
                w2tt[[slot_expert[c][s] for s in range(NSL)]]),
            "tile_e": np.tile(te[None, :], (128, 1)),
            "tile_lo": np.tile(tlo[None, :], (128, 1)),
            "rhl": rhl,
            "iota42m2": iota42m2,
            "iota128r": iota128r,
            "ident": ident,
            "uts128": uts128,
            "rep16": rep16,
            "sel8": sel8,
            "hz": np.ascontiguousarray(hidden_rows[c * 256:(c + 1) * 256]),
            "seqidx": (seq_base + c * 256).astype(np.int16),
        })
    return in_maps, t_of_r


def kernel(hidden_states, router_w, e_score_correction_bias, w13, w2,
           _trace=False):
    nc = build_nc()
    in_maps, t_of_r = make_in_maps(hidden_states, router_w,
                                   e_score_correction_bias, w13, w2)
    res = run_bass_kernel_spmd(nc, in_maps, core_ids=list(range(NCORES)),
                               trace=_trace)
    total = np.zeros((T, H), np.float64)
    for c in range(NCORES):
        total += res.results[c]["partial"].astype(np.float64)
    out = np.empty((T, H), np.float32)
    out[t_of_r] = total.astype(np.float32)      # out[t] = total[r(t)]
    kernel._last_results = res
    return out
